# revision 21
# baseline (speedup 1.0000x reference)
"""Trainium2 Bass kernel for the MultiHeadAttention problem.

Math (per head h):
  scores = (X Wq_h) (X Wk_h)^T * scale = X (scale * Wq_h Wk_h^T) X^T
so we precompute M_h = (scale*Wq_h) Wk_h^T once per head (batch independent),
then per batch compute scores directly in the transposed [m, n] orientation so
softmax's reduction axis (m) lands on partitions and A feeds the A@V matmul
without transposes:
  TT[d', n] = sum_d M[d, d'] X^T[d, n]
  scoresT[m, n] = sum_d' X^T[d', m] TT[d', n]
  A = exp(scoresT - colmax) -> HhT[v, n] = sum_m V[m, v] A[m, n] / colsum
Output projection: Y_partial = concatT^T @ Wo_local, with the post-hoc triu
mask filled with LARGE_NEG/8 on every core so the host-side shard-sum yields
exactly LARGE_NEG at masked positions.

Sharding: 16 heads / 8 cores = 2 heads per core, every core handles all 4
batches; host sums the 8 partial outputs (the only cross-core reduction).

Precision: the score path (M, TT, scoresT) and V use single-pass fp32r
matmuls — TRN2's fp32r mode runs at bf16 rate (1 cycle/row for moving dim
>= 256) with ~12-bit mantissa operand precision, giving score errors ~0.3
absolute against softmax top-2 gaps of ~400, so argmax fidelity is preserved
without the old 3-pass bf16 hi/lo splits.  V is computed transposed
(VT = Wv2^T X^T, both heads stacked, moving dim 512) then PE-transposed into
key-major layout to avoid overhead-dominated free=64 matmuls.  The A/V/AV and
output-projection paths run in 1-pass bf16 (~0.5% relative, far inside
tolerance).
"""

import os
import sys

import numpy as np
import ml_dtypes

for _p in ("/opt/trn_rl_repo",):
    if os.path.isdir(_p) and _p not in sys.path:
        sys.path.insert(0, _p)

import concourse.bass as bass
import concourse.tile as tile
from concourse import bacc, bass_isa, mybir

BF = mybir.dt.bfloat16
F32 = mybir.dt.float32
F32R = mybir.dt.float32r
bf16 = ml_dtypes.bfloat16

# Problem constants (hardcoded per contract)
B, N, D, DV, H = 4, 1024, 1024, 64, 16
NCORES = 8
HLOC = H // NCORES  # heads per core
P = 128
FREE = 512  # PSUM free-dim limit for fp32 outputs
LARGE_NEG = -1.0e9


def _fsplits(total, step):
    return [(o, min(step, total - o)) for o in range(0, total, step)]


def build_mha_body(tc, ins, y_ap, b_sz=B, n_sz=N, d_sz=D, dv=DV, hloc=HLOC,
                   fill_div=NCORES):
    """Emit the per-core MHA program into TileContext tc.

    ins: dict of dram APs (f32r unless noted): xt [b, d, n], wqt/wkt
    [hloc, d, d] (wqt pre-scaled), wv2 [d, hloc*dv], wo [hloc*dv, d] (bf16).
    y_ap: [b, n, d] f32 output.
    """
    nc = tc.nc
    nch_d = d_sz // P
    nch_n = n_sz // P
    half = min(FREE, n_sz)
    hv = hloc * dv
    assert hv <= P
    fill = float(LARGE_NEG / fill_div)

    import contextlib
    ctx = contextlib.ExitStack()
    with ctx:
        p_m = ctx.enter_context(tc.tile_pool(name="mpool", bufs=1))
        p_xt = ctx.enter_context(tc.tile_pool(name="xt", bufs=1))
        p_wk = ctx.enter_context(tc.tile_pool(name="wk", bufs=1))
        p_wq = ctx.enter_context(tc.tile_pool(name="wq", bufs=2))
        p_tt = ctx.enter_context(tc.tile_pool(name="tt", bufs=1))
        p_sraw = ctx.enter_context(tc.tile_pool(name="sraw", bufs=1))
        p_a = ctx.enter_context(tc.tile_pool(name="apool", bufs=1))
        p_v = ctx.enter_context(tc.tile_pool(name="vpool", bufs=1))
        p_wv = ctx.enter_context(tc.tile_pool(name="wv", bufs=1))
        p_wo = ctx.enter_context(tc.tile_pool(name="wo", bufs=1))
        p_cat = ctx.enter_context(tc.tile_pool(name="cat", bufs=1))
        p_misc1 = ctx.enter_context(tc.tile_pool(name="misc1", bufs=1))
        p_y = ctx.enter_context(tc.tile_pool(name="yout", bufs=1))
        ps = ctx.enter_context(tc.tile_pool(name="ps", bufs=4, space="PSUM"))
        ps_v = ctx.enter_context(tc.tile_pool(name="psv", bufs=1, space="PSUM"))
        ps_o = ctx.enter_context(tc.tile_pool(name="pso", bufs=3, space="PSUM"))

        # Wo resident (local head rows), natural layout [hv, d], bf16
        wo = p_wo.tile([hv, d_sz], BF, tag="wo")
        nc.sync.dma_start(wo[:], ins["wo"][:])
        # Wv for both local heads stacked column-wise: [P, nch_d, hv] f32r
        wv2 = p_wv.tile([P, nch_d, hv], F32R, tag="wv2")
        nc.sync.dma_start(wv2[:], ins["wv2"].rearrange("(c p) v -> p c v", p=P))
        fill_tile = p_wo.tile([P, FREE // 2], F32, tag="fill_tile")
        nc.gpsimd.memset(fill_tile[:], fill)
        # identity for PE transposes: start from ones, keep only the diagonal
        ident = p_wo.tile([P, P], F32, tag="ident")
        nc.gpsimd.memset(ident[:], 1.0)
        nc.gpsimd.affine_select(
            out=ident[:], in_=ident[:], compare_op=mybir.AluOpType.is_equal,
            fill=0.0, base=0, pattern=[[-1, P]], channel_multiplier=1)

        concat_tiles = {}
        vb_tiles = {}
        pending = []  # deferred AV/out-proj emitters, flushed after the next
        # slab's TT matmuls so the PE never waits on a softmax chain

        # visit order over (hl, b): snake so head boundaries reuse X^T
        visits = []
        for hl in range(hloc):
            border = range(b_sz) if hl % 2 == 0 else range(b_sz - 1, -1, -1)
            visits.extend((hl, b) for b in border)
        xt_seq = []  # deduped consecutive batch sequence (positions)
        vis_pos = []
        for _, b in visits:
            if not xt_seq or xt_seq[-1] != b:
                xt_seq.append(b)
            vis_pos.append(len(xt_seq) - 1)
        # explicit double-buffering: position i lives in buffer i%2, which
        # under the snake order exactly matches buffer liveness
        xt_state = {}  # buf -> owning batch
        xt_live = {}   # batch -> tile

        def xt_load(pos):
            b = xt_seq[pos]
            buf = pos % 2
            if xt_state.get(buf) == b:
                return xt_live[b]
            t = p_xt.tile([P, nch_d, n_sz], F32R, tag=f"xt{buf}",
                          name=f"xt{b}p{pos}")
            for c in range(nch_d):
                nc.sync.dma_start(t[:, c, :],
                                  ins["xt"][b][c * P:(c + 1) * P, :])
            old = xt_state.get(buf)
            if old is not None:
                xt_live.pop(old, None)
            xt_state[buf] = b
            xt_live[b] = t
            return t

        wkf_tiles = {}

        def ensure_wkf(h):
            if h in wkf_tiles:
                return wkf_tiles[h]
            t = p_wk.tile([P, nch_d, d_sz], F32R, tag="wkf", name=f"wkf{h}")
            for c in range(nch_d):
                nc.sync.dma_start(t[:, c, :],
                                  ins["wkt"][h][c * P:(c + 1) * P, :])
            wkf_tiles[h] = t
            return t

        vi = 0
        for hl in range(hloc):
            # ---- M phase: M[d, d'] = sum_e WqT[e, d] WkT[e, d'] (f32r)
            # Wk fully resident per head; each weight byte is DMA'd exactly once.
            m_t = p_m.tile([P, nch_d, d_sz], F32R, tag="m")
            wkf = ensure_wkf(hl)
            for dc in range(nch_d):
                wq = p_wq.tile([P, nch_d, P], F32R, tag="wq")
                nc.sync.dma_start(
                    wq[:],
                    ins["wqt"][hl][:, dc * P:(dc + 1) * P].rearrange(
                        "(c p) f -> p c f", p=P))
                for (dpo, dps) in _fsplits(d_sz, FREE):
                    pst = ps.tile([P, FREE], F32, tag="ps")
                    for e in range(nch_d):
                        nc.tensor.matmul(pst[:, :dps], wq[:, e, :],
                                         wkf[:, e, dpo:dpo + dps],
                                         start=(e == 0), stop=(e == nch_d - 1))
                    nc.scalar.copy(m_t[:, dc, dpo:dpo + dps], pst[:, :dps])

            # ---- attention phase (snake order so the head boundary reuses
            # the resident X^T tile of the last batch)
            border = range(b_sz) if hl % 2 == 0 else range(b_sz - 1, -1, -1)
            for b in border:
                pos = vis_pos[vi]
                vi += 1
                xt = xt_load(pos)
                # prefetch the next position's X^T so its DMA dispatches
                # before this batch's long compute phase
                if pos + 1 < len(xt_seq):
                    xt_load(pos + 1)
                if b == (border[-1] if hl % 2 == 0 else 0) and hl + 1 < hloc:
                    ensure_wkf(hl + 1)

                if b not in concat_tiles:
                    concat_tiles[b] = p_cat.tile([hv, n_sz], BF, tag=f"cat{b}",
                                                 name=f"cat{b}")
                cat = concat_tiles[b]

                # ---- V for both heads, once per batch (at first head):
                # VT[v2, n] = sum_d Wv2[d, v2] X^T[d, n]  (f32r, moving dim 512)
                # then PE-transpose 128x128 blocks into key-major v_both (bf16)
                if b not in vb_tiles:
                    vb_tiles[b] = p_v.tile([P, nch_n, hv], BF, tag=f"vb{b}",
                                           name=f"vb{b}")
                    vb = vb_tiles[b]
                    for (nho, nhs) in _fsplits(n_sz, half):
                        pvt = ps_v.tile([P, FREE], F32, tag="psv")
                        for c in range(nch_d):
                            nc.tensor.matmul(pvt[:hv, :nhs], wv2[:, c, :],
                                             xt[:, c, nho:nho + nhs],
                                             start=(c == 0), stop=(c == nch_d - 1))
                        vt_sb = p_misc1.tile([P, FREE], F32, tag="s1", name="vt_sb")
                        nc.vector.tensor_copy(vt_sb[:hv, :nhs], pvt[:hv, :nhs])
                        for j in range(nhs // P):
                            ptr = ps_o.tile([P, FREE], F32, tag="pstr")
                            nc.tensor.transpose(
                                ptr[:, :hv], vt_sb[:hv, j * P:(j + 1) * P],
                                ident[:hv, :hv])
                            nc.vector.tensor_copy(vb[:, nho // P + j, :],
                                                  ptr[:, :hv])
                v_both = vb_tiles[b]

                for (nho, nhs) in _fsplits(n_sz, half):
                    # TT[d', n-half] = sum_d M[d, d'] XT[d, n]  (f32r)
                    tt = p_tt.tile([P, nch_d, half], F32R, tag="tt")
                    for dp in range(nch_d):
                        pst = ps.tile([P, FREE], F32, tag="ps")
                        for dc in range(nch_d):
                            nc.tensor.matmul(
                                pst[:, :nhs], m_t[:, dc, dp * P:(dp + 1) * P],
                                xt[:, dc, nho:nho + nhs],
                                start=(dc == 0), stop=(dc == nch_d - 1))
                        if dp % 2 == 0:
                            nc.vector.tensor_copy(tt[:, dp, :nhs], pst[:, :nhs])
                        else:
                            nc.scalar.copy(tt[:, dp, :nhs], pst[:, :nhs])
                        if dp == 1:
                            # fire the pending flush's long-latency prologue
                            # (reciprocal) here: behind the first two tt casts
                            # in the DVE queue but ~a slab ahead of its use
                            for pre, _fn in pending:
                                if pre is not None:
                                    pre()

                    for _pre, fn in pending:
                        fn()
                    pending.clear()

                    # scoresT[m, n-half]  (f32r)
                    sraw = p_sraw.tile([P, nch_n, half], F32, tag="sraw")
                    runmax = p_misc1.tile([P, half], F32, tag="runmax")
                    for mc in range(nch_n):
                        pst = ps.tile([P, FREE], F32, tag="ps")
                        for c in range(nch_d):
                            nc.tensor.matmul(
                                pst[:, :nhs], xt[:, c, mc * P:(mc + 1) * P],
                                tt[:, c, :nhs],
                                start=(c == 0), stop=(c == nch_d - 1))
                        nc.scalar.copy(sraw[:, mc, :nhs], pst[:, :nhs])
                        if mc == 0:
                            nc.vector.tensor_copy(runmax[:, :nhs], sraw[:, 0, :nhs])
                        else:
                            nc.vector.tensor_max(runmax[:, :nhs], runmax[:, :nhs],
                                                 sraw[:, mc, :nhs])

                    # softmax over m (partition axis x chunk axis)
                    maxb = p_misc1.tile([P, half], F32, tag="maxb")
                    nc.gpsimd.partition_all_reduce(maxb[:, :nhs], runmax[:, :nhs], P,
                                                   bass_isa.ReduceOp.max)
                    a_t = p_a.tile([P, nch_n, half], BF, tag="a")
                    s1 = p_misc1.tile([P, half], F32, tag="s1")
                    for mc in range(nch_n):
                        nc.vector.tensor_sub(sraw[:, mc, :nhs], sraw[:, mc, :nhs],
                                             maxb[:, :nhs])
                        nc.scalar.activation(a_t[:, mc, :nhs], sraw[:, mc, :nhs],
                                             mybir.ActivationFunctionType.Exp)
                        if mc == 0:
                            nc.vector.tensor_copy(s1[:, :nhs], a_t[:, 0, :nhs])
                        else:
                            nc.vector.tensor_add(s1[:, :nhs], s1[:, :nhs],
                                                 a_t[:, mc, :nhs])
                    denb = p_misc1.tile([P, half], F32, tag="maxb", name="denb")
                    nc.gpsimd.partition_all_reduce(denb[:, :nhs], s1[:, :nhs], P,
                                                   bass_isa.ReduceOp.add)

                    # HhT[v, n-half] = sum_m V[m, v] A[m, n] -- deferred (bf16)
                    # recip prologue fires mid-TT-slab (after the 2nd cast) so
                    # it neither blocks the tt casts nor delays the cat mult
                    rbox = {}

                    def emit_recip(denb=denb, nhs=nhs, rbox=rbox):
                        recip = p_misc1.tile([P, half], F32, tag="runmax",
                                             name="recip")
                        nc.vector.reciprocal(recip[:dv, :nhs], denb[:dv, :nhs])
                        rbox["recip"] = recip

                    def emit_av(v_both=v_both, a_t=a_t, cat=cat, hl=hl,
                                nho=nho, nhs=nhs, rbox=rbox):
                        psav = ps_v.tile([P, FREE], F32, tag="psv", name="psav")
                        for mc in range(nch_n):
                            nc.tensor.matmul(
                                psav[:dv, :nhs],
                                v_both[:, mc, hl * dv:(hl + 1) * dv],
                                a_t[:, mc, :nhs],
                                start=(mc == 0), stop=(mc == nch_n - 1))
                        recip = rbox["recip"]
                        nc.vector.tensor_mul(cat[hl * dv:(hl + 1) * dv, nho:nho + nhs],
                                             psav[:dv, :nhs], recip[:dv, :nhs])
                    pending.append((emit_recip, emit_av))

                    # ---- output projection rows for this half once all
                    # heads are done (pipelines under the next half's TT)
                    if hl == hloc - 1:
                        def emit_outproj(cat=cat, b=b, nho=nho, nhs=nhs):
                            for ncc in range(nho // P, (nho + nhs) // P):
                                ct = cat[:, ncc * P:(ncc + 1) * P]
                                for (dho, dhs) in _fsplits(d_sz, FREE):
                                    if dho >= ncc * P + P:
                                        # fully masked block: constant fill
                                        for fo in range(0, dhs, FREE // 2):
                                            fs = min(FREE // 2, dhs - fo)
                                            nc.sync.dma_start(
                                                y_ap[b, ncc * P:(ncc + 1) * P,
                                                     dho + fo:dho + fo + fs],
                                                fill_tile[:, :fs])
                                        continue
                                    pst = ps_o.tile([P, FREE], F32, tag="pstr",
                                                    name="psy")
                                    nc.tensor.matmul(pst[:, :dhs], ct,
                                                     wo[:, dho:dho + dhs],
                                                     start=True, stop=True)
                                    # drain in 256-wide pieces alternating two
                                    # small yt tiles and two copy engines so
                                    # nothing idles on the copy->select->DMA
                                    # WAR chain; fully-masked pieces skip the
                                    # copy entirely (constant fill DMA)
                                    for fo in range(0, dhs, FREE // 2):
                                        fs = min(FREE // 2, dhs - fo)
                                        if dho + fo > ncc * P + P - 1:
                                            nc.sync.dma_start(
                                                y_ap[b, ncc * P:(ncc + 1) * P,
                                                     dho + fo:dho + fo + fs],
                                                fill_tile[:, :fs])
                                            continue
                                        pidx = (dho + fo) // (FREE // 2)
                                        yt = p_y.tile([P, FREE // 2], F32,
                                                      tag=f"yt{pidx % 2}",
                                                      name=f"yt{pidx % 2}")
                                        if pidx % 2 == 0:
                                            nc.scalar.copy(yt[:, :fs],
                                                           pst[:, fo:fo + fs])
                                        else:
                                            nc.vector.tensor_copy(
                                                yt[:, :fs], pst[:, fo:fo + fs])
                                        if dho + fo + fs > ncc * P + 1:
                                            # diagonal piece: keep where
                                            # row - col >= 0, else fill
                                            nc.gpsimd.affine_select(
                                                out=yt[:, :fs], in_=yt[:, :fs],
                                                compare_op=mybir.AluOpType.is_ge,
                                                fill=fill,
                                                base=ncc * P - dho - fo,
                                                pattern=[[-1, fs]],
                                                channel_multiplier=1)
                                        nc.sync.dma_start(
                                            y_ap[b, ncc * P:(ncc + 1) * P,
                                                 dho + fo:dho + fo + fs],
                                            yt[:, :fs])
                        pending.append((None, emit_outproj))

        for pre, fn in pending:
            if pre is not None:
                pre()
            fn()
        pending.clear()


def build_program(b_sz=B, n_sz=N, d_sz=D, dv=DV, hloc=HLOC, fill_div=NCORES,
                  num_devices=NCORES):
    nc = bacc.Bacc("TRN2", target_bir_lowering=False, debug=False,
                   num_devices=num_devices)
    hv = hloc * dv
    specs = {
        "xt": ([b_sz, d_sz, n_sz], F32R),
        "wqt": ([hloc, d_sz, d_sz], F32R),
        "wkt": ([hloc, d_sz, d_sz], F32R),
        "wv2": ([d_sz, hv], F32R),
        "wo": ([hv, d_sz], BF),
    }
    ins = {k: nc.dram_tensor(k, shp, dt, kind="ExternalInput").ap()
           for k, (shp, dt) in specs.items()}
    y = nc.dram_tensor("y", [b_sz, n_sz, d_sz], F32, kind="ExternalOutput").ap()
    with tile.TileContext(nc) as tc:
        build_mha_body(tc, ins, y, b_sz=b_sz, n_sz=n_sz, d_sz=d_sz, dv=dv,
                       hloc=hloc, fill_div=fill_div)
    nc.compile()
    return nc


def make_in_maps(X, W_q, W_k, W_v, W_o, ncores=NCORES, hloc=HLOC):
    scale = np.float32(1.0 / np.sqrt(X.shape[2]))
    xt = np.ascontiguousarray(X.transpose(0, 2, 1))
    dvv = W_v.shape[2]
    in_maps = []
    for c in range(ncores):
        hs = slice(c * hloc, (c + 1) * hloc)
        wqt = np.ascontiguousarray((W_q[hs] * scale).transpose(0, 2, 1))
        wkt = np.ascontiguousarray(W_k[hs].transpose(0, 2, 1))
        wv2 = np.ascontiguousarray(
            np.concatenate([W_v[c * hloc + i] for i in range(hloc)], axis=1))
        wo = np.ascontiguousarray(
            W_o[c * hloc * dvv:(c + 1) * hloc * dvv]).astype(bf16)
        in_maps.append({
            "xt": xt, "wqt": wqt, "wkt": wkt, "wv2": wv2, "wo": wo,
        })
    return in_maps


_CACHE = {}


def kernel(X, W_q, W_k, W_v, W_o, _trace=False):
    from concourse.bass_utils import run_bass_kernel_spmd
    X = np.asarray(X, dtype=np.float32)
    W_q = np.asarray(W_q, dtype=np.float32)
    W_k = np.asarray(W_k, dtype=np.float32)
    W_v = np.asarray(W_v, dtype=np.float32)
    W_o = np.asarray(W_o, dtype=np.float32)

    if "nc" not in _CACHE:
        _CACHE["nc"] = build_program()
    nc = _CACHE["nc"]

    in_maps = make_in_maps(X, W_q, W_k, W_v, W_o)
    res = run_bass_kernel_spmd(nc, in_maps, list(range(NCORES)), trace=_trace)
    parts = [r["y"].astype(np.float32) for r in res.results]
    out = parts[0]
    for p in parts[1:]:
        out = out + p
    if _trace:
        _CACHE["last_result"] = res
    return out


# revision 23
# speedup vs baseline: 1.0316x; 1.0316x over previous
"""Trainium2 Bass kernel for the MultiHeadAttention problem.

Math (per head h):
  scores = (X Wq_h) (X Wk_h)^T * scale = X (scale * Wq_h Wk_h^T) X^T
so we precompute M_h = (scale*Wq_h) Wk_h^T once per head (batch independent),
then per batch compute scores directly in the transposed [m, n] orientation so
softmax's reduction axis (m) lands on partitions and A feeds the A@V matmul
without transposes:
  TT[d', n] = sum_d M[d, d'] X^T[d, n]
  scoresT[m, n] = sum_d' X^T[d', m] TT[d', n]
  A = exp(scoresT - colmax) -> HhT[v, n] = sum_m V[m, v] A[m, n] / colsum
Output projection: Y_partial = concatT^T @ Wo_local, with the post-hoc triu
mask filled with LARGE_NEG/8 on every core so the host-side shard-sum yields
exactly LARGE_NEG at masked positions.

Sharding: 16 heads / 8 cores = 2 heads per core, every core handles all 4
batches; host sums the 8 partial outputs (the only cross-core reduction).

Precision: the score path (M, TT, scoresT) and V use single-pass fp32r
matmuls — TRN2's fp32r mode runs at bf16 rate (1 cycle/row for moving dim
>= 256) with ~12-bit mantissa operand precision, giving score errors ~0.3
absolute against softmax top-2 gaps of ~400, so argmax fidelity is preserved
without the old 3-pass bf16 hi/lo splits.  V is computed transposed
(VT = Wv2^T X^T, both heads stacked, moving dim 512) then PE-transposed into
key-major layout to avoid overhead-dominated free=64 matmuls.  The A/V/AV and
output-projection paths run in 1-pass bf16 (~0.5% relative, far inside
tolerance).
"""

import os
import sys

import numpy as np
import ml_dtypes

for _p in ("/opt/trn_rl_repo",):
    if os.path.isdir(_p) and _p not in sys.path:
        sys.path.insert(0, _p)

import concourse.bass as bass
import concourse.tile as tile
from concourse import bacc, bass_isa, mybir

BF = mybir.dt.bfloat16
F32 = mybir.dt.float32
F32R = mybir.dt.float32r
bf16 = ml_dtypes.bfloat16

# Problem constants (hardcoded per contract)
B, N, D, DV, H = 4, 1024, 1024, 64, 16
NCORES = 8
HLOC = H // NCORES  # heads per core
P = 128
FREE = 512  # PSUM free-dim limit for fp32 outputs
LARGE_NEG = -1.0e9


def _fsplits(total, step):
    return [(o, min(step, total - o)) for o in range(0, total, step)]


def build_mha_body(tc, ins, y_ap, b_sz=B, n_sz=N, d_sz=D, dv=DV, hloc=HLOC,
                   fill_div=NCORES):
    """Emit the per-core MHA program into TileContext tc.

    ins: dict of dram APs (f32r unless noted): xt [b, d, n], wqt/wkt
    [hloc, d, d] (wqt pre-scaled), wv2 [d, hloc*dv], wo [hloc*dv, d] (bf16).
    y_ap: [b, n, d] f32 output.
    """
    nc = tc.nc
    nch_d = d_sz // P
    nch_n = n_sz // P
    half = min(FREE, n_sz)
    hv = hloc * dv
    assert hv <= P
    fill = float(LARGE_NEG / fill_div)

    import contextlib
    ctx = contextlib.ExitStack()
    with ctx:
        p_m = ctx.enter_context(tc.tile_pool(name="mpool", bufs=1))
        p_xt = ctx.enter_context(tc.tile_pool(name="xt", bufs=1))
        p_wk = ctx.enter_context(tc.tile_pool(name="wk", bufs=1))
        p_wq = ctx.enter_context(tc.tile_pool(name="wq", bufs=2))
        p_tt = ctx.enter_context(tc.tile_pool(name="tt", bufs=1))
        p_sraw = ctx.enter_context(tc.tile_pool(name="sraw", bufs=1))
        p_a = ctx.enter_context(tc.tile_pool(name="apool", bufs=1))
        p_v = ctx.enter_context(tc.tile_pool(name="vpool", bufs=1))
        p_wv = ctx.enter_context(tc.tile_pool(name="wv", bufs=1))
        p_wo = ctx.enter_context(tc.tile_pool(name="wo", bufs=1))
        p_cat = ctx.enter_context(tc.tile_pool(name="cat", bufs=1))
        p_misc1 = ctx.enter_context(tc.tile_pool(name="misc1", bufs=1))
        p_y = ctx.enter_context(tc.tile_pool(name="yout", bufs=1))
        ps = ctx.enter_context(tc.tile_pool(name="ps", bufs=4, space="PSUM"))
        ps_v = ctx.enter_context(tc.tile_pool(name="psv", bufs=1, space="PSUM"))
        ps_o = ctx.enter_context(tc.tile_pool(name="pso", bufs=3, space="PSUM"))

        # Wo resident (local head rows), natural layout [hv, d], bf16
        wo = p_wo.tile([hv, d_sz], BF, tag="wo")
        nc.sync.dma_start(wo[:], ins["wo"][:])
        # Wv for both local heads stacked column-wise: [P, nch_d, hv] f32r
        wv2 = p_wv.tile([P, nch_d, hv], F32R, tag="wv2")
        nc.sync.dma_start(wv2[:], ins["wv2"].rearrange("(c p) v -> p c v", p=P))
        fill_tile = p_wo.tile([P, FREE // 2], F32, tag="fill_tile")
        nc.gpsimd.memset(fill_tile[:], fill)
        # identity for PE transposes: start from ones, keep only the diagonal
        ident = p_wo.tile([P, P], F32, tag="ident")
        nc.gpsimd.memset(ident[:], 1.0)
        nc.gpsimd.affine_select(
            out=ident[:], in_=ident[:], compare_op=mybir.AluOpType.is_equal,
            fill=0.0, base=0, pattern=[[-1, P]], channel_multiplier=1)

        concat_tiles = {}
        vb_tiles = {}
        pending = []  # deferred AV/out-proj emitters, flushed after the next
        # slab's TT matmuls so the PE never waits on a softmax chain

        # visit order over (hl, b): snake so head boundaries reuse X^T
        visits = []
        for hl in range(hloc):
            border = range(b_sz) if hl % 2 == 0 else range(b_sz - 1, -1, -1)
            visits.extend((hl, b) for b in border)
        xt_seq = []  # deduped consecutive batch sequence (positions)
        vis_pos = []
        for _, b in visits:
            if not xt_seq or xt_seq[-1] != b:
                xt_seq.append(b)
            vis_pos.append(len(xt_seq) - 1)
        # explicit double-buffering: position i lives in buffer i%2, which
        # under the snake order exactly matches buffer liveness
        xt_state = {}  # buf -> owning batch
        xt_live = {}   # batch -> tile

        def xt_load(pos):
            b = xt_seq[pos]
            buf = pos % 2
            if xt_state.get(buf) == b:
                return xt_live[b]
            t = p_xt.tile([P, nch_d, n_sz], F32R, tag=f"xt{buf}",
                          name=f"xt{b}p{pos}")
            for c in range(nch_d):
                nc.sync.dma_start(t[:, c, :],
                                  ins["xt"][b][c * P:(c + 1) * P, :])
            old = xt_state.get(buf)
            if old is not None:
                xt_live.pop(old, None)
            xt_state[buf] = b
            xt_live[b] = t
            return t

        wkf_tiles = {}

        def ensure_wkf(h):
            if h in wkf_tiles:
                return wkf_tiles[h]
            t = p_wk.tile([P, nch_d, d_sz], F32R, tag="wkf", name=f"wkf{h}")
            for c in range(nch_d):
                nc.sync.dma_start(t[:, c, :],
                                  ins["wkt"][h][c * P:(c + 1) * P, :])
            wkf_tiles[h] = t
            return t

        vi = 0
        for hl in range(hloc):
            # ---- M phase: M[d, d'] = sum_e WqT[e, d] WkT[e, d'] (f32r)
            # Wk fully resident per head; each weight byte is DMA'd exactly once.
            m_t = p_m.tile([P, nch_d, d_sz], F32R, tag="m")
            wkf = ensure_wkf(hl)
            for dc in range(nch_d):
                wq = p_wq.tile([P, nch_d, P], F32R, tag="wq")
                nc.sync.dma_start(
                    wq[:],
                    ins["wqt"][hl][:, dc * P:(dc + 1) * P].rearrange(
                        "(c p) f -> p c f", p=P))
                for (dpo, dps) in _fsplits(d_sz, FREE):
                    pst = ps.tile([P, FREE], F32, tag="ps")
                    for e in range(nch_d):
                        nc.tensor.matmul(pst[:, :dps], wq[:, e, :],
                                         wkf[:, e, dpo:dpo + dps],
                                         start=(e == 0), stop=(e == nch_d - 1))
                    nc.scalar.copy(m_t[:, dc, dpo:dpo + dps], pst[:, :dps])

            # ---- attention phase (snake order so the head boundary reuses
            # the resident X^T tile of the last batch)
            border = range(b_sz) if hl % 2 == 0 else range(b_sz - 1, -1, -1)
            for b in border:
                pos = vis_pos[vi]
                vi += 1
                xt = xt_load(pos)
                # prefetch the next position's X^T so its DMA dispatches
                # before this batch's long compute phase
                if pos + 1 < len(xt_seq):
                    xt_load(pos + 1)
                if b == (border[-1] if hl % 2 == 0 else 0) and hl + 1 < hloc:
                    ensure_wkf(hl + 1)

                if b not in concat_tiles:
                    concat_tiles[b] = p_cat.tile([hv, n_sz], BF, tag=f"cat{b}",
                                                 name=f"cat{b}")
                cat = concat_tiles[b]

                # ---- V for both heads, once per batch (at first head):
                # VT[v2, n] = sum_d Wv2[d, v2] X^T[d, n]  (f32r, moving dim 512)
                # then PE-transpose 128x128 blocks into key-major v_both (bf16)
                if b not in vb_tiles:
                    vb_tiles[b] = p_v.tile([P, nch_n, hv], BF, tag=f"vb{b}",
                                           name=f"vb{b}")
                    vb = vb_tiles[b]
                    for (nho, nhs) in _fsplits(n_sz, half):
                        pvt = ps_v.tile([P, FREE], F32, tag="psv")
                        for c in range(nch_d):
                            nc.tensor.matmul(pvt[:hv, :nhs], wv2[:, c, :],
                                             xt[:, c, nho:nho + nhs],
                                             start=(c == 0), stop=(c == nch_d - 1))
                        vt_sb = p_misc1.tile([P, FREE], F32, tag="s1", name="vt_sb")
                        nc.vector.tensor_copy(vt_sb[:hv, :nhs], pvt[:hv, :nhs])
                        for j in range(nhs // P):
                            ptr = ps_o.tile([P, FREE], F32, tag="pstr")
                            nc.tensor.transpose(
                                ptr[:, :hv], vt_sb[:hv, j * P:(j + 1) * P],
                                ident[:hv, :hv])
                            nc.vector.tensor_copy(vb[:, nho // P + j, :],
                                                  ptr[:, :hv])
                v_both = vb_tiles[b]

                for (nho, nhs) in _fsplits(n_sz, half):
                    # TT[d', n-half] = sum_d M[d, d'] XT[d, n]  (f32r)
                    tt = p_tt.tile([P, nch_d, half], F32R, tag="tt")
                    for dp in range(nch_d):
                        pst = ps.tile([P, FREE], F32, tag="ps")
                        for dc in range(nch_d):
                            nc.tensor.matmul(
                                pst[:, :nhs], m_t[:, dc, dp * P:(dp + 1) * P],
                                xt[:, dc, nho:nho + nhs],
                                start=(dc == 0), stop=(dc == nch_d - 1))
                        if dp % 2 == 0:
                            nc.vector.tensor_copy(tt[:, dp, :nhs], pst[:, :nhs])
                        else:
                            nc.scalar.copy(tt[:, dp, :nhs], pst[:, :nhs])

                    # scoresT[m, n-half]  (f32r)
                    sraw = p_sraw.tile([P, nch_n, half], F32, tag="sraw")
                    runmax = p_misc1.tile([P, half], F32, tag="runmax")
                    for mc in range(nch_n):
                        pst = ps.tile([P, FREE], F32, tag="ps")
                        for c in range(nch_d):
                            nc.tensor.matmul(
                                pst[:, :nhs], xt[:, c, mc * P:(mc + 1) * P],
                                tt[:, c, :nhs],
                                start=(c == 0), stop=(c == nch_d - 1))
                        nc.scalar.copy(sraw[:, mc, :nhs], pst[:, :nhs])
                        if mc == 0:
                            nc.vector.tensor_copy(runmax[:, :nhs], sraw[:, 0, :nhs])
                        else:
                            nc.vector.tensor_max(runmax[:, :nhs], runmax[:, :nhs],
                                                 sraw[:, mc, :nhs])
                        if mc == 3:
                            # flush the previous half's deferred AV/outproj
                            # here: its softmax chain has had a full TT slab
                            # plus half this scores sweep to complete
                            for pre, fn in pending:
                                if pre is not None:
                                    pre()
                                fn()
                            pending.clear()

                    # softmax over m (partition axis x chunk axis)
                    maxb = p_misc1.tile([P, half], F32, tag="maxb")
                    nc.gpsimd.partition_all_reduce(maxb[:, :nhs], runmax[:, :nhs], P,
                                                   bass_isa.ReduceOp.max)
                    a_t = p_a.tile([P, nch_n, half], BF, tag="a")
                    s1 = p_misc1.tile([P, half], F32, tag="s1")
                    for mc in range(nch_n):
                        nc.vector.tensor_sub(sraw[:, mc, :nhs], sraw[:, mc, :nhs],
                                             maxb[:, :nhs])
                        nc.scalar.activation(a_t[:, mc, :nhs], sraw[:, mc, :nhs],
                                             mybir.ActivationFunctionType.Exp)
                        if mc == 0:
                            nc.vector.tensor_copy(s1[:, :nhs], a_t[:, 0, :nhs])
                        else:
                            nc.vector.tensor_add(s1[:, :nhs], s1[:, :nhs],
                                                 a_t[:, mc, :nhs])
                    denb = p_misc1.tile([P, half], F32, tag="maxb", name="denb")
                    nc.gpsimd.partition_all_reduce(denb[:, :nhs], s1[:, :nhs], P,
                                                   bass_isa.ReduceOp.add)

                    # HhT[v, n-half] = sum_m V[m, v] A[m, n] -- deferred (bf16)
                    # recip prologue fires mid-TT-slab (after the 2nd cast) so
                    # it neither blocks the tt casts nor delays the cat mult
                    rbox = {}

                    def emit_recip(denb=denb, nhs=nhs, rbox=rbox):
                        recip = p_misc1.tile([P, half], F32, tag="s1",
                                             name="recip")
                        nc.vector.reciprocal(recip[:dv, :nhs], denb[:dv, :nhs])
                        rbox["recip"] = recip

                    def emit_av(v_both=v_both, a_t=a_t, cat=cat, hl=hl,
                                nho=nho, nhs=nhs, rbox=rbox):
                        psav = ps_v.tile([P, FREE], F32, tag="psv", name="psav")
                        for mc in range(nch_n):
                            nc.tensor.matmul(
                                psav[:dv, :nhs],
                                v_both[:, mc, hl * dv:(hl + 1) * dv],
                                a_t[:, mc, :nhs],
                                start=(mc == 0), stop=(mc == nch_n - 1))
                        recip = rbox["recip"]
                        nc.vector.tensor_mul(cat[hl * dv:(hl + 1) * dv, nho:nho + nhs],
                                             psav[:dv, :nhs], recip[:dv, :nhs])
                    pending.append((emit_recip, emit_av))

                    # ---- output projection rows for this half once all
                    # heads are done (pipelines under the next half's TT)
                    if hl == hloc - 1:
                        def emit_outproj(cat=cat, b=b, nho=nho, nhs=nhs):
                            for ncc in range(nho // P, (nho + nhs) // P):
                                ct = cat[:, ncc * P:(ncc + 1) * P]
                                for (dho, dhs) in _fsplits(d_sz, FREE):
                                    if dho >= ncc * P + P:
                                        # fully masked block: constant fill
                                        for fo in range(0, dhs, FREE // 2):
                                            fs = min(FREE // 2, dhs - fo)
                                            nc.sync.dma_start(
                                                y_ap[b, ncc * P:(ncc + 1) * P,
                                                     dho + fo:dho + fo + fs],
                                                fill_tile[:, :fs])
                                        continue
                                    pst = ps_o.tile([P, FREE], F32, tag="pstr",
                                                    name="psy")
                                    nc.tensor.matmul(pst[:, :dhs], ct,
                                                     wo[:, dho:dho + dhs],
                                                     start=True, stop=True)
                                    # drain in 256-wide pieces alternating two
                                    # small yt tiles and two copy engines so
                                    # nothing idles on the copy->select->DMA
                                    # WAR chain; fully-masked pieces skip the
                                    # copy entirely (constant fill DMA)
                                    for fo in range(0, dhs, FREE // 2):
                                        fs = min(FREE // 2, dhs - fo)
                                        if dho + fo > ncc * P + P - 1:
                                            nc.sync.dma_start(
                                                y_ap[b, ncc * P:(ncc + 1) * P,
                                                     dho + fo:dho + fo + fs],
                                                fill_tile[:, :fs])
                                            continue
                                        pidx = (dho + fo) // (FREE // 2)
                                        yt = p_y.tile([P, FREE // 2], F32,
                                                      tag=f"yt{pidx % 2}",
                                                      name=f"yt{pidx % 2}")
                                        if pidx % 2 == 0:
                                            nc.scalar.copy(yt[:, :fs],
                                                           pst[:, fo:fo + fs])
                                        else:
                                            nc.vector.tensor_copy(
                                                yt[:, :fs], pst[:, fo:fo + fs])
                                        if dho + fo + fs > ncc * P + 1:
                                            # diagonal piece: keep where
                                            # row - col >= 0, else fill
                                            nc.gpsimd.affine_select(
                                                out=yt[:, :fs], in_=yt[:, :fs],
                                                compare_op=mybir.AluOpType.is_ge,
                                                fill=fill,
                                                base=ncc * P - dho - fo,
                                                pattern=[[-1, fs]],
                                                channel_multiplier=1)
                                        nc.sync.dma_start(
                                            y_ap[b, ncc * P:(ncc + 1) * P,
                                                 dho + fo:dho + fo + fs],
                                            yt[:, :fs])
                        pending.append((None, emit_outproj))

        for pre, fn in pending:
            if pre is not None:
                pre()
            fn()
        pending.clear()


def build_program(b_sz=B, n_sz=N, d_sz=D, dv=DV, hloc=HLOC, fill_div=NCORES,
                  num_devices=NCORES):
    nc = bacc.Bacc("TRN2", target_bir_lowering=False, debug=False,
                   num_devices=num_devices)
    hv = hloc * dv
    specs = {
        "xt": ([b_sz, d_sz, n_sz], F32R),
        "wqt": ([hloc, d_sz, d_sz], F32R),
        "wkt": ([hloc, d_sz, d_sz], F32R),
        "wv2": ([d_sz, hv], F32R),
        "wo": ([hv, d_sz], BF),
    }
    ins = {k: nc.dram_tensor(k, shp, dt, kind="ExternalInput").ap()
           for k, (shp, dt) in specs.items()}
    y = nc.dram_tensor("y", [b_sz, n_sz, d_sz], F32, kind="ExternalOutput").ap()
    with tile.TileContext(nc) as tc:
        build_mha_body(tc, ins, y, b_sz=b_sz, n_sz=n_sz, d_sz=d_sz, dv=dv,
                       hloc=hloc, fill_div=fill_div)
    nc.compile()
    return nc


def make_in_maps(X, W_q, W_k, W_v, W_o, ncores=NCORES, hloc=HLOC):
    scale = np.float32(1.0 / np.sqrt(X.shape[2]))
    xt = np.ascontiguousarray(X.transpose(0, 2, 1))
    dvv = W_v.shape[2]
    in_maps = []
    for c in range(ncores):
        hs = slice(c * hloc, (c + 1) * hloc)
        wqt = np.ascontiguousarray((W_q[hs] * scale).transpose(0, 2, 1))
        wkt = np.ascontiguousarray(W_k[hs].transpose(0, 2, 1))
        wv2 = np.ascontiguousarray(
            np.concatenate([W_v[c * hloc + i] for i in range(hloc)], axis=1))
        wo = np.ascontiguousarray(
            W_o[c * hloc * dvv:(c + 1) * hloc * dvv]).astype(bf16)
        in_maps.append({
            "xt": xt, "wqt": wqt, "wkt": wkt, "wv2": wv2, "wo": wo,
        })
    return in_maps


_CACHE = {}


def kernel(X, W_q, W_k, W_v, W_o, _trace=False):
    from concourse.bass_utils import run_bass_kernel_spmd
    X = np.asarray(X, dtype=np.float32)
    W_q = np.asarray(W_q, dtype=np.float32)
    W_k = np.asarray(W_k, dtype=np.float32)
    W_v = np.asarray(W_v, dtype=np.float32)
    W_o = np.asarray(W_o, dtype=np.float32)

    if "nc" not in _CACHE:
        _CACHE["nc"] = build_program()
    nc = _CACHE["nc"]

    in_maps = make_in_maps(X, W_q, W_k, W_v, W_o)
    res = run_bass_kernel_spmd(nc, in_maps, list(range(NCORES)), trace=_trace)
    parts = [r["y"].astype(np.float32) for r in res.results]
    out = parts[0]
    for p in parts[1:]:
        out = out + p
    if _trace:
        _CACHE["last_result"] = res
    return out


# revision 24
# speedup vs baseline: 1.0810x; 1.0479x over previous
"""Trainium2 Bass kernel for the MultiHeadAttention problem.

Math (per head h):
  scores = (X Wq_h) (X Wk_h)^T * scale = X (scale * Wq_h Wk_h^T) X^T
so we precompute M_h = (scale*Wq_h) Wk_h^T once per head (batch independent),
then per batch compute scores directly in the transposed [m, n] orientation so
softmax's reduction axis (m) lands on partitions and A feeds the A@V matmul
without transposes:
  TT[d', n] = sum_d M[d, d'] X^T[d, n]
  scoresT[m, n] = sum_d' X^T[d', m] TT[d', n]
  A = exp(scoresT - colmax) -> HhT[v, n] = sum_m V[m, v] A[m, n] / colsum
Output projection: Y_partial = concatT^T @ Wo_local, with the post-hoc triu
mask filled with LARGE_NEG/8 on every core so the host-side shard-sum yields
exactly LARGE_NEG at masked positions.

Sharding: 16 heads / 8 cores = 2 heads per core, every core handles all 4
batches; host sums the 8 partial outputs (the only cross-core reduction).

Precision: the score path (M, TT, scoresT) and V use single-pass fp32r
matmuls — TRN2's fp32r mode runs at bf16 rate (1 cycle/row for moving dim
>= 256) with ~12-bit mantissa operand precision, giving score errors ~0.3
absolute against softmax top-2 gaps of ~400, so argmax fidelity is preserved
without the old 3-pass bf16 hi/lo splits.  V is computed transposed
(VT = Wv2^T X^T, both heads stacked, moving dim 512) then PE-transposed into
key-major layout to avoid overhead-dominated free=64 matmuls.  The A/V/AV and
output-projection paths run in 1-pass bf16 (~0.5% relative, far inside
tolerance).
"""

import os
import sys

import numpy as np
import ml_dtypes

for _p in ("/opt/trn_rl_repo",):
    if os.path.isdir(_p) and _p not in sys.path:
        sys.path.insert(0, _p)

import concourse.bass as bass
import concourse.tile as tile
from concourse import bacc, bass_isa, mybir

BF = mybir.dt.bfloat16
F32 = mybir.dt.float32
F32R = mybir.dt.float32r
bf16 = ml_dtypes.bfloat16

# Problem constants (hardcoded per contract)
B, N, D, DV, H = 4, 1024, 1024, 64, 16
NCORES = 8
HLOC = H // NCORES  # heads per core
P = 128
FREE = 512  # PSUM free-dim limit for fp32 outputs
LARGE_NEG = -1.0e9


def _fsplits(total, step):
    return [(o, min(step, total - o)) for o in range(0, total, step)]


def build_mha_body(tc, ins, y_ap, b_sz=B, n_sz=N, d_sz=D, dv=DV, hloc=HLOC,
                   fill_div=NCORES):
    """Emit the per-core MHA program into TileContext tc.

    ins: dict of dram APs (f32r unless noted): xt [b, d, n], wqt/wkt
    [hloc, d, d] (wqt pre-scaled), wv2 [d, hloc*dv], wo [hloc*dv, d] (bf16).
    y_ap: [b, n, d] f32 output.
    """
    nc = tc.nc
    nch_d = d_sz // P
    nch_n = n_sz // P
    half = min(FREE, n_sz)
    hv = hloc * dv
    assert hv <= P
    fill = float(LARGE_NEG / fill_div)

    import contextlib
    ctx = contextlib.ExitStack()
    with ctx:
        p_m = ctx.enter_context(tc.tile_pool(name="mpool", bufs=1))
        p_xt = ctx.enter_context(tc.tile_pool(name="xt", bufs=1))
        p_wk = ctx.enter_context(tc.tile_pool(name="wk", bufs=1))
        p_wq = ctx.enter_context(tc.tile_pool(name="wq", bufs=2))
        p_tt = ctx.enter_context(tc.tile_pool(name="tt", bufs=1))
        p_sraw = ctx.enter_context(tc.tile_pool(name="sraw", bufs=1))
        p_a = ctx.enter_context(tc.tile_pool(name="apool", bufs=1))
        p_v = ctx.enter_context(tc.tile_pool(name="vpool", bufs=1))
        p_wv = ctx.enter_context(tc.tile_pool(name="wv", bufs=1))
        p_wo = ctx.enter_context(tc.tile_pool(name="wo", bufs=1))
        p_cat = ctx.enter_context(tc.tile_pool(name="cat", bufs=1))
        p_misc1 = ctx.enter_context(tc.tile_pool(name="misc1", bufs=1))
        p_y = ctx.enter_context(tc.tile_pool(name="yout", bufs=1))
        ps = ctx.enter_context(tc.tile_pool(name="ps", bufs=4, space="PSUM"))
        ps_v = ctx.enter_context(tc.tile_pool(name="psv", bufs=1, space="PSUM"))
        ps_o = ctx.enter_context(tc.tile_pool(name="pso", bufs=3, space="PSUM"))

        # Wo resident (local head rows), natural layout [hv, d], bf16
        wo = p_wo.tile([hv, d_sz], BF, tag="wo")
        nc.sync.dma_start(wo[:], ins["wo"][:])
        # Wv for both local heads stacked column-wise: [P, nch_d, hv] f32r
        wv2 = p_wv.tile([P, nch_d, hv], F32R, tag="wv2")
        nc.sync.dma_start(wv2[:], ins["wv2"].rearrange("(c p) v -> p c v", p=P))
        fill_tile = p_wo.tile([P, FREE // 2], F32, tag="fill_tile")
        nc.gpsimd.memset(fill_tile[:], fill)
        # identity for PE transposes: start from ones, keep only the diagonal
        ident = p_wo.tile([P, P], F32, tag="ident")
        nc.gpsimd.memset(ident[:], 1.0)
        nc.gpsimd.affine_select(
            out=ident[:], in_=ident[:], compare_op=mybir.AluOpType.is_equal,
            fill=0.0, base=0, pattern=[[-1, P]], channel_multiplier=1)

        concat_tiles = {}
        vb_tiles = {}
        pending = []  # deferred AV/out-proj emitters, flushed after the next
        # slab's TT matmuls so the PE never waits on a softmax chain

        # visit order over (hl, b): snake so head boundaries reuse X^T
        visits = []
        for hl in range(hloc):
            border = range(b_sz) if hl % 2 == 0 else range(b_sz - 1, -1, -1)
            visits.extend((hl, b) for b in border)
        xt_seq = []  # deduped consecutive batch sequence (positions)
        vis_pos = []
        for _, b in visits:
            if not xt_seq or xt_seq[-1] != b:
                xt_seq.append(b)
            vis_pos.append(len(xt_seq) - 1)
        # explicit double-buffering: position i lives in buffer i%2, which
        # under the snake order exactly matches buffer liveness
        xt_state = {}  # buf -> owning batch
        xt_live = {}   # batch -> tile

        def xt_load(pos):
            b = xt_seq[pos]
            buf = pos % 2
            if xt_state.get(buf) == b:
                return xt_live[b]
            t = p_xt.tile([P, nch_d, n_sz], F32R, tag=f"xt{buf}",
                          name=f"xt{b}p{pos}")
            for c in range(nch_d):
                nc.sync.dma_start(t[:, c, :],
                                  ins["xt"][b][c * P:(c + 1) * P, :])
            old = xt_state.get(buf)
            if old is not None:
                xt_live.pop(old, None)
            xt_state[buf] = b
            xt_live[b] = t
            return t

        wkf_tiles = {}

        def ensure_wkf(h):
            if h in wkf_tiles:
                return wkf_tiles[h]
            t = p_wk.tile([P, nch_d, d_sz], F32R, tag="wkf", name=f"wkf{h}")
            for c in range(nch_d):
                nc.sync.dma_start(t[:, c, :],
                                  ins["wkt"][h][c * P:(c + 1) * P, :])
            wkf_tiles[h] = t
            return t

        vi = 0
        for hl in range(hloc):
            # ---- M phase: M[d, d'] = sum_e WqT[e, d] WkT[e, d'] (f32r)
            # Wk fully resident per head; each weight byte is DMA'd exactly once.
            m_t = p_m.tile([P, nch_d, d_sz], F32R, tag="m")
            wkf = ensure_wkf(hl)
            for dc in range(nch_d):
                wq = p_wq.tile([P, nch_d, P], F32R, tag="wq")
                nc.sync.dma_start(
                    wq[:],
                    ins["wqt"][hl][:, dc * P:(dc + 1) * P].rearrange(
                        "(c p) f -> p c f", p=P))
                for (dpo, dps) in _fsplits(d_sz, FREE):
                    pst = ps.tile([P, FREE], F32, tag="ps")
                    for e in range(nch_d):
                        nc.tensor.matmul(pst[:, :dps], wq[:, e, :],
                                         wkf[:, e, dpo:dpo + dps],
                                         start=(e == 0), stop=(e == nch_d - 1))
                    nc.scalar.copy(m_t[:, dc, dpo:dpo + dps], pst[:, :dps])

            # ---- attention phase (snake order so the head boundary reuses
            # the resident X^T tile of the last batch)
            border = range(b_sz) if hl % 2 == 0 else range(b_sz - 1, -1, -1)
            for b in border:
                pos = vis_pos[vi]
                vi += 1
                xt = xt_load(pos)
                # prefetch the next position's X^T so its DMA dispatches
                # before this batch's long compute phase
                if pos + 1 < len(xt_seq):
                    xt_load(pos + 1)
                if b == (border[-1] if hl % 2 == 0 else 0) and hl + 1 < hloc:
                    ensure_wkf(hl + 1)

                if b not in concat_tiles:
                    concat_tiles[b] = p_cat.tile([hv, n_sz], BF, tag=f"cat{b}",
                                                 name=f"cat{b}")
                cat = concat_tiles[b]

                # ---- V for both heads, once per batch (at first head):
                # VT[v2, n] = sum_d Wv2[d, v2] X^T[d, n]  (f32r, moving dim 512)
                # then PE-transpose 128x128 blocks into key-major v_both (bf16)
                if b not in vb_tiles:
                    vb_tiles[b] = p_v.tile([P, nch_n, hv], BF, tag=f"vb{b}",
                                           name=f"vb{b}")
                    vb = vb_tiles[b]
                    for (nho, nhs) in _fsplits(n_sz, half):
                        pvt = ps_v.tile([P, FREE], F32, tag="psv")
                        for c in range(nch_d):
                            nc.tensor.matmul(pvt[:hv, :nhs], wv2[:, c, :],
                                             xt[:, c, nho:nho + nhs],
                                             start=(c == 0), stop=(c == nch_d - 1))
                        vt_sb = p_misc1.tile([P, FREE], F32, tag="s1", name="vt_sb")
                        nc.vector.tensor_copy(vt_sb[:hv, :nhs], pvt[:hv, :nhs])
                        for j in range(nhs // P):
                            ptr = ps_o.tile([P, FREE], F32, tag="pstr")
                            nc.tensor.transpose(
                                ptr[:, :hv], vt_sb[:hv, j * P:(j + 1) * P],
                                ident[:hv, :hv])
                            nc.vector.tensor_copy(vb[:, nho // P + j, :],
                                                  ptr[:, :hv])
                v_both = vb_tiles[b]

                # the very last visit runs its halves high-to-low so the
                # final (uncovered) outproj flush is the cheap ncc 0-3 one
                halves = _fsplits(n_sz, half)
                if vi == len(visits):
                    halves = halves[::-1]
                for (nho, nhs) in halves:
                    # TT[d', n-half] = sum_d M[d, d'] XT[d, n]  (f32r)
                    tt = p_tt.tile([P, nch_d, half], F32R, tag="tt")
                    for dp in range(nch_d):
                        pst = ps.tile([P, FREE], F32, tag="ps")
                        for dc in range(nch_d):
                            nc.tensor.matmul(
                                pst[:, :nhs], m_t[:, dc, dp * P:(dp + 1) * P],
                                xt[:, dc, nho:nho + nhs],
                                start=(dc == 0), stop=(dc == nch_d - 1))
                        if dp % 2 == 0:
                            nc.vector.tensor_copy(tt[:, dp, :nhs], pst[:, :nhs])
                        else:
                            nc.scalar.copy(tt[:, dp, :nhs], pst[:, :nhs])

                    # scoresT[m, n-half]  (f32r)
                    sraw = p_sraw.tile([P, nch_n, half], F32, tag="sraw")
                    runmax = p_misc1.tile([P, half], F32, tag="runmax")
                    for mc in range(nch_n):
                        pst = ps.tile([P, FREE], F32, tag="ps")
                        for c in range(nch_d):
                            nc.tensor.matmul(
                                pst[:, :nhs], xt[:, c, mc * P:(mc + 1) * P],
                                tt[:, c, :nhs],
                                start=(c == 0), stop=(c == nch_d - 1))
                        nc.scalar.copy(sraw[:, mc, :nhs], pst[:, :nhs])
                        if mc == 0:
                            nc.vector.tensor_copy(runmax[:, :nhs], sraw[:, 0, :nhs])
                        else:
                            nc.vector.tensor_max(runmax[:, :nhs], runmax[:, :nhs],
                                                 sraw[:, mc, :nhs])
                        if mc == 3:
                            # flush the previous half's deferred AV/outproj
                            # here: its softmax chain has had a full TT slab
                            # plus half this scores sweep to complete
                            for pre, fn in pending:
                                if pre is not None:
                                    pre()
                                fn()
                            pending.clear()

                    # softmax over m (partition axis x chunk axis)
                    maxb = p_misc1.tile([P, half], F32, tag="maxb")
                    nc.gpsimd.partition_all_reduce(maxb[:, :nhs], runmax[:, :nhs], P,
                                                   bass_isa.ReduceOp.max)
                    a_t = p_a.tile([P, nch_n, half], BF, tag="a")
                    s1 = p_misc1.tile([P, half], F32, tag="s1")
                    for mc in range(nch_n):
                        nc.vector.tensor_sub(sraw[:, mc, :nhs], sraw[:, mc, :nhs],
                                             maxb[:, :nhs])
                        nc.scalar.activation(a_t[:, mc, :nhs], sraw[:, mc, :nhs],
                                             mybir.ActivationFunctionType.Exp)
                        if mc == 0:
                            nc.vector.tensor_copy(s1[:, :nhs], a_t[:, 0, :nhs])
                        else:
                            nc.vector.tensor_add(s1[:, :nhs], s1[:, :nhs],
                                                 a_t[:, mc, :nhs])
                    denb = p_misc1.tile([P, half], F32, tag="maxb", name="denb")
                    nc.gpsimd.partition_all_reduce(denb[:, :nhs], s1[:, :nhs], P,
                                                   bass_isa.ReduceOp.add)

                    # HhT[v, n-half] = sum_m V[m, v] A[m, n] -- deferred (bf16)
                    # recip prologue fires mid-TT-slab (after the 2nd cast) so
                    # it neither blocks the tt casts nor delays the cat mult
                    rbox = {}

                    def emit_recip(denb=denb, nhs=nhs, rbox=rbox):
                        recip = p_misc1.tile([P, half], F32, tag="s1",
                                             name="recip")
                        nc.vector.reciprocal(recip[:dv, :nhs], denb[:dv, :nhs])
                        rbox["recip"] = recip

                    def emit_av(v_both=v_both, a_t=a_t, cat=cat, hl=hl,
                                nho=nho, nhs=nhs, rbox=rbox):
                        psav = ps_v.tile([P, FREE], F32, tag="psv", name="psav")
                        for mc in range(nch_n):
                            nc.tensor.matmul(
                                psav[:dv, :nhs],
                                v_both[:, mc, hl * dv:(hl + 1) * dv],
                                a_t[:, mc, :nhs],
                                start=(mc == 0), stop=(mc == nch_n - 1))
                        recip = rbox["recip"]
                        nc.vector.tensor_mul(cat[hl * dv:(hl + 1) * dv, nho:nho + nhs],
                                             psav[:dv, :nhs], recip[:dv, :nhs])
                    pending.append((emit_recip, emit_av))

                    # ---- output projection rows for this half once all
                    # heads are done (pipelines under the next half's TT)
                    if hl == hloc - 1:
                        def emit_outproj(cat=cat, b=b, nho=nho, nhs=nhs):
                            for ncc in range(nho // P, (nho + nhs) // P):
                                ct = cat[:, ncc * P:(ncc + 1) * P]
                                for (dho, dhs) in _fsplits(d_sz, FREE):
                                    if dho >= ncc * P + P:
                                        # fully masked block: constant fill
                                        for fo in range(0, dhs, FREE // 2):
                                            fs = min(FREE // 2, dhs - fo)
                                            nc.sync.dma_start(
                                                y_ap[b, ncc * P:(ncc + 1) * P,
                                                     dho + fo:dho + fo + fs],
                                                fill_tile[:, :fs])
                                        continue
                                    pst = ps_o.tile([P, FREE], F32, tag="pstr",
                                                    name="psy")
                                    nc.tensor.matmul(pst[:, :dhs], ct,
                                                     wo[:, dho:dho + dhs],
                                                     start=True, stop=True)
                                    # drain in 256-wide pieces alternating two
                                    # small yt tiles and two copy engines so
                                    # nothing idles on the copy->select->DMA
                                    # WAR chain; fully-masked pieces skip the
                                    # copy entirely (constant fill DMA)
                                    for fo in range(0, dhs, FREE // 2):
                                        fs = min(FREE // 2, dhs - fo)
                                        if dho + fo > ncc * P + P - 1:
                                            nc.sync.dma_start(
                                                y_ap[b, ncc * P:(ncc + 1) * P,
                                                     dho + fo:dho + fo + fs],
                                                fill_tile[:, :fs])
                                            continue
                                        pidx = (dho + fo) // (FREE // 2)
                                        yt = p_y.tile([P, FREE // 2], F32,
                                                      tag=f"yt{pidx % 2}",
                                                      name=f"yt{pidx % 2}")
                                        if pidx % 2 == 0:
                                            nc.scalar.copy(yt[:, :fs],
                                                           pst[:, fo:fo + fs])
                                        else:
                                            nc.vector.tensor_copy(
                                                yt[:, :fs], pst[:, fo:fo + fs])
                                        if dho + fo + fs > ncc * P + 1:
                                            # diagonal piece: keep where
                                            # row - col >= 0, else fill
                                            nc.gpsimd.affine_select(
                                                out=yt[:, :fs], in_=yt[:, :fs],
                                                compare_op=mybir.AluOpType.is_ge,
                                                fill=fill,
                                                base=ncc * P - dho - fo,
                                                pattern=[[-1, fs]],
                                                channel_multiplier=1)
                                        nc.sync.dma_start(
                                            y_ap[b, ncc * P:(ncc + 1) * P,
                                                 dho + fo:dho + fo + fs],
                                            yt[:, :fs])
                        pending.append((None, emit_outproj))

        for pre, fn in pending:
            if pre is not None:
                pre()
            fn()
        pending.clear()


def build_program(b_sz=B, n_sz=N, d_sz=D, dv=DV, hloc=HLOC, fill_div=NCORES,
                  num_devices=NCORES):
    nc = bacc.Bacc("TRN2", target_bir_lowering=False, debug=False,
                   num_devices=num_devices)
    hv = hloc * dv
    specs = {
        "xt": ([b_sz, d_sz, n_sz], F32R),
        "wqt": ([hloc, d_sz, d_sz], F32R),
        "wkt": ([hloc, d_sz, d_sz], F32R),
        "wv2": ([d_sz, hv], F32R),
        "wo": ([hv, d_sz], BF),
    }
    ins = {k: nc.dram_tensor(k, shp, dt, kind="ExternalInput").ap()
           for k, (shp, dt) in specs.items()}
    y = nc.dram_tensor("y", [b_sz, n_sz, d_sz], F32, kind="ExternalOutput").ap()
    with tile.TileContext(nc) as tc:
        build_mha_body(tc, ins, y, b_sz=b_sz, n_sz=n_sz, d_sz=d_sz, dv=dv,
                       hloc=hloc, fill_div=fill_div)
    nc.compile()
    return nc


def make_in_maps(X, W_q, W_k, W_v, W_o, ncores=NCORES, hloc=HLOC):
    scale = np.float32(1.0 / np.sqrt(X.shape[2]))
    xt = np.ascontiguousarray(X.transpose(0, 2, 1))
    dvv = W_v.shape[2]
    in_maps = []
    for c in range(ncores):
        hs = slice(c * hloc, (c + 1) * hloc)
        wqt = np.ascontiguousarray((W_q[hs] * scale).transpose(0, 2, 1))
        wkt = np.ascontiguousarray(W_k[hs].transpose(0, 2, 1))
        wv2 = np.ascontiguousarray(
            np.concatenate([W_v[c * hloc + i] for i in range(hloc)], axis=1))
        wo = np.ascontiguousarray(
            W_o[c * hloc * dvv:(c + 1) * hloc * dvv]).astype(bf16)
        in_maps.append({
            "xt": xt, "wqt": wqt, "wkt": wkt, "wv2": wv2, "wo": wo,
        })
    return in_maps


_CACHE = {}


def kernel(X, W_q, W_k, W_v, W_o, _trace=False):
    from concourse.bass_utils import run_bass_kernel_spmd
    X = np.asarray(X, dtype=np.float32)
    W_q = np.asarray(W_q, dtype=np.float32)
    W_k = np.asarray(W_k, dtype=np.float32)
    W_v = np.asarray(W_v, dtype=np.float32)
    W_o = np.asarray(W_o, dtype=np.float32)

    if "nc" not in _CACHE:
        _CACHE["nc"] = build_program()
    nc = _CACHE["nc"]

    in_maps = make_in_maps(X, W_q, W_k, W_v, W_o)
    res = run_bass_kernel_spmd(nc, in_maps, list(range(NCORES)), trace=_trace)
    parts = [r["y"].astype(np.float32) for r in res.results]
    out = parts[0]
    for p in parts[1:]:
        out = out + p
    if _trace:
        _CACHE["last_result"] = res
    return out


# revision 25
# speedup vs baseline: 1.0985x; 1.0162x over previous
"""Trainium2 Bass kernel for the MultiHeadAttention problem.

Math (per head h):
  scores = (X Wq_h) (X Wk_h)^T * scale = X (scale * Wq_h Wk_h^T) X^T
so we precompute M_h = (scale*Wq_h) Wk_h^T once per head (batch independent),
then per batch compute scores directly in the transposed [m, n] orientation so
softmax's reduction axis (m) lands on partitions and A feeds the A@V matmul
without transposes:
  TT[d', n] = sum_d M[d, d'] X^T[d, n]
  scoresT[m, n] = sum_d' X^T[d', m] TT[d', n]
  A = exp(scoresT - colmax) -> HhT[v, n] = sum_m V[m, v] A[m, n] / colsum
Output projection: Y_partial = concatT^T @ Wo_local, with the post-hoc triu
mask filled with LARGE_NEG/8 on every core so the host-side shard-sum yields
exactly LARGE_NEG at masked positions.

Sharding: 16 heads / 8 cores = 2 heads per core, every core handles all 4
batches; host sums the 8 partial outputs (the only cross-core reduction).

Precision: the score path (M, TT, scoresT) and V use single-pass fp32r
matmuls — TRN2's fp32r mode runs at bf16 rate (1 cycle/row for moving dim
>= 256) with ~12-bit mantissa operand precision, giving score errors ~0.3
absolute against softmax top-2 gaps of ~400, so argmax fidelity is preserved
without the old 3-pass bf16 hi/lo splits.  V is computed transposed
(VT = Wv2^T X^T, both heads stacked, moving dim 512) then PE-transposed into
key-major layout to avoid overhead-dominated free=64 matmuls.  The A/V/AV and
output-projection paths run in 1-pass bf16 (~0.5% relative, far inside
tolerance).
"""

import os
import sys

import numpy as np
import ml_dtypes

for _p in ("/opt/trn_rl_repo",):
    if os.path.isdir(_p) and _p not in sys.path:
        sys.path.insert(0, _p)

import concourse.bass as bass
import concourse.tile as tile
from concourse import bacc, bass_isa, mybir

BF = mybir.dt.bfloat16
F32 = mybir.dt.float32
F32R = mybir.dt.float32r
bf16 = ml_dtypes.bfloat16

# Problem constants (hardcoded per contract)
B, N, D, DV, H = 4, 1024, 1024, 64, 16
NCORES = 8
HLOC = H // NCORES  # heads per core
P = 128
FREE = 512  # PSUM free-dim limit for fp32 outputs
LARGE_NEG = -1.0e9


def _fsplits(total, step):
    return [(o, min(step, total - o)) for o in range(0, total, step)]


def build_mha_body(tc, ins, y_ap, b_sz=B, n_sz=N, d_sz=D, dv=DV, hloc=HLOC,
                   fill_div=NCORES):
    """Emit the per-core MHA program into TileContext tc.

    ins: dict of dram APs (f32r unless noted): xt [b, d, n], wqt/wkt
    [hloc, d, d] (wqt pre-scaled), wv2 [d, hloc*dv], wo [hloc*dv, d] (bf16).
    y_ap: [b, n, d] f32 output.
    """
    nc = tc.nc
    nch_d = d_sz // P
    nch_n = n_sz // P
    half = min(FREE, n_sz)
    hv = hloc * dv
    assert hv <= P
    fill = float(LARGE_NEG / fill_div)

    import contextlib
    ctx = contextlib.ExitStack()
    with ctx:
        p_m = ctx.enter_context(tc.tile_pool(name="mpool", bufs=1))
        p_xt = ctx.enter_context(tc.tile_pool(name="xt", bufs=1))
        p_wk = ctx.enter_context(tc.tile_pool(name="wk", bufs=1))
        p_wq = ctx.enter_context(tc.tile_pool(name="wq", bufs=2))
        p_tt = ctx.enter_context(tc.tile_pool(name="tt", bufs=1))
        p_sraw = ctx.enter_context(tc.tile_pool(name="sraw", bufs=1))
        p_a = ctx.enter_context(tc.tile_pool(name="apool", bufs=1))
        p_v = ctx.enter_context(tc.tile_pool(name="vpool", bufs=1))
        p_wv = ctx.enter_context(tc.tile_pool(name="wv", bufs=1))
        p_wo = ctx.enter_context(tc.tile_pool(name="wo", bufs=1))
        p_cat = ctx.enter_context(tc.tile_pool(name="cat", bufs=1))
        p_misc1 = ctx.enter_context(tc.tile_pool(name="misc1", bufs=1))
        p_y = ctx.enter_context(tc.tile_pool(name="yout", bufs=1))
        ps = ctx.enter_context(tc.tile_pool(name="ps", bufs=4, space="PSUM"))
        ps_v = ctx.enter_context(tc.tile_pool(name="psv", bufs=1, space="PSUM"))
        ps_o = ctx.enter_context(tc.tile_pool(name="pso", bufs=3, space="PSUM"))

        # Wo resident (local head rows), natural layout [hv, d], bf16
        wo = p_wo.tile([hv, d_sz], BF, tag="wo")
        nc.sync.dma_start(wo[:], ins["wo"][:])
        # Wv for both local heads stacked column-wise: [P, nch_d, hv] f32r
        wv2 = p_wv.tile([P, nch_d, hv], F32R, tag="wv2")
        nc.sync.dma_start(wv2[:], ins["wv2"].rearrange("(c p) v -> p c v", p=P))
        fill_tile = p_wo.tile([P, FREE // 2], F32, tag="fill_tile")
        nc.gpsimd.memset(fill_tile[:], fill)
        # identity for PE transposes: start from ones, keep only the diagonal
        ident = p_wo.tile([P, P], F32, tag="ident")
        nc.gpsimd.memset(ident[:], 1.0)
        nc.gpsimd.affine_select(
            out=ident[:], in_=ident[:], compare_op=mybir.AluOpType.is_equal,
            fill=0.0, base=0, pattern=[[-1, P]], channel_multiplier=1)

        concat_tiles = {}
        vb_tiles = {}
        pending = []  # deferred AV/out-proj emitters, flushed after the next
        # slab's TT matmuls so the PE never waits on a softmax chain

        # visit order over (hl, b): snake so head boundaries reuse X^T
        visits = []
        for hl in range(hloc):
            border = range(b_sz) if hl % 2 == 0 else range(b_sz - 1, -1, -1)
            visits.extend((hl, b) for b in border)
        xt_seq = []  # deduped consecutive batch sequence (positions)
        vis_pos = []
        for _, b in visits:
            if not xt_seq or xt_seq[-1] != b:
                xt_seq.append(b)
            vis_pos.append(len(xt_seq) - 1)
        # explicit double-buffering: position i lives in buffer i%2, which
        # under the snake order exactly matches buffer liveness
        xt_state = {}  # buf -> owning batch
        xt_live = {}   # batch -> tile

        def xt_load(pos):
            b = xt_seq[pos]
            buf = pos % 2
            if xt_state.get(buf) == b:
                return xt_live[b]
            t = p_xt.tile([P, nch_d, n_sz], F32R, tag=f"xt{buf}",
                          name=f"xt{b}p{pos}")
            for c in range(nch_d):
                nc.sync.dma_start(t[:, c, :],
                                  ins["xt"][b][c * P:(c + 1) * P, :])
            old = xt_state.get(buf)
            if old is not None:
                xt_live.pop(old, None)
            xt_state[buf] = b
            xt_live[b] = t
            return t

        wkf_tiles = {}

        def ensure_wkf(h):
            if h in wkf_tiles:
                return wkf_tiles[h]
            t = p_wk.tile([P, nch_d, d_sz], F32R, tag="wkf", name=f"wkf{h}")
            for c in range(nch_d):
                nc.sync.dma_start(t[:, c, :],
                                  ins["wkt"][h][c * P:(c + 1) * P, :])
            wkf_tiles[h] = t
            return t

        vi = 0
        for hl in range(hloc):
            # ---- M phase: M[d, d'] = sum_e WqT[e, d] WkT[e, d'] (f32r)
            # Wk fully resident per head; each weight byte is DMA'd exactly once.
            m_t = p_m.tile([P, nch_d, d_sz], F32R, tag="m")
            wkf = ensure_wkf(hl)
            for dc in range(nch_d):
                wq = p_wq.tile([P, nch_d, P], F32R, tag="wq")
                nc.sync.dma_start(
                    wq[:],
                    ins["wqt"][hl][:, dc * P:(dc + 1) * P].rearrange(
                        "(c p) f -> p c f", p=P))
                for (dpo, dps) in _fsplits(d_sz, FREE):
                    pst = ps.tile([P, FREE], F32, tag="ps")
                    for e in range(nch_d):
                        nc.tensor.matmul(pst[:, :dps], wq[:, e, :],
                                         wkf[:, e, dpo:dpo + dps],
                                         start=(e == 0), stop=(e == nch_d - 1))
                    nc.scalar.copy(m_t[:, dc, dpo:dpo + dps], pst[:, :dps])

            # ---- attention phase (snake order so the head boundary reuses
            # the resident X^T tile of the last batch)
            border = range(b_sz) if hl % 2 == 0 else range(b_sz - 1, -1, -1)
            for b in border:
                pos = vis_pos[vi]
                vi += 1
                xt = xt_load(pos)
                # prefetch the next position's X^T so its DMA dispatches
                # before this batch's long compute phase
                if pos + 1 < len(xt_seq):
                    xt_load(pos + 1)
                if b == (border[-1] if hl % 2 == 0 else 0) and hl + 1 < hloc:
                    ensure_wkf(hl + 1)

                if b not in concat_tiles:
                    concat_tiles[b] = p_cat.tile([hv, n_sz], BF, tag=f"cat{b}",
                                                 name=f"cat{b}")
                cat = concat_tiles[b]

                # ---- V for both heads, once per batch (at first head):
                # VT[v2, n] = sum_d Wv2[d, v2] X^T[d, n]  (f32r, moving dim 512)
                # then PE-transpose 128x128 blocks into key-major v_both (bf16).
                # Emission is deferred until after the first TT slab so its PE
                # work covers the previous half's softmax chain and its DVE
                # copies queue behind the tt casts.
                emit_v = None
                if b not in vb_tiles:
                    vb_tiles[b] = p_v.tile([P, nch_n, hv], BF, tag=f"vb{b}",
                                           name=f"vb{b}")

                    def emit_v(vb=vb_tiles[b], xt=xt):
                        for (vnho, vnhs) in _fsplits(n_sz, half):
                            pvt = ps_v.tile([P, FREE], F32, tag="psv")
                            for c in range(nch_d):
                                nc.tensor.matmul(pvt[:hv, :vnhs], wv2[:, c, :],
                                                 xt[:, c, vnho:vnho + vnhs],
                                                 start=(c == 0),
                                                 stop=(c == nch_d - 1))
                            vt_sb = p_misc1.tile([P, FREE], F32, tag="s1",
                                                 name="vt_sb")
                            nc.vector.tensor_copy(vt_sb[:hv, :vnhs],
                                                  pvt[:hv, :vnhs])
                            for j in range(vnhs // P):
                                ptr = ps_o.tile([P, FREE], F32, tag="pstr")
                                nc.tensor.transpose(
                                    ptr[:, :hv],
                                    vt_sb[:hv, j * P:(j + 1) * P],
                                    ident[:hv, :hv])
                                nc.vector.tensor_copy(vb[:, vnho // P + j, :],
                                                      ptr[:, :hv])
                v_both = vb_tiles[b]

                # the very last visit runs its halves high-to-low so the
                # final (uncovered) outproj flush is the cheap ncc 0-3 one
                halves = _fsplits(n_sz, half)
                if vi == len(visits):
                    halves = halves[::-1]
                for (nho, nhs) in halves:
                    # TT[d', n-half] = sum_d M[d, d'] XT[d, n]  (f32r)
                    tt = p_tt.tile([P, nch_d, half], F32R, tag="tt")
                    for dp in range(nch_d):
                        pst = ps.tile([P, FREE], F32, tag="ps")
                        for dc in range(nch_d):
                            nc.tensor.matmul(
                                pst[:, :nhs], m_t[:, dc, dp * P:(dp + 1) * P],
                                xt[:, dc, nho:nho + nhs],
                                start=(dc == 0), stop=(dc == nch_d - 1))
                        if dp % 2 == 0:
                            nc.vector.tensor_copy(tt[:, dp, :nhs], pst[:, :nhs])
                        else:
                            nc.scalar.copy(tt[:, dp, :nhs], pst[:, :nhs])

                    if emit_v is not None:
                        emit_v()
                        emit_v = None

                    # scoresT[m, n-half]  (f32r)
                    sraw = p_sraw.tile([P, nch_n, half], F32, tag="sraw")
                    runmax = p_misc1.tile([P, half], F32, tag="runmax")
                    for mc in range(nch_n):
                        pst = ps.tile([P, FREE], F32, tag="ps")
                        for c in range(nch_d):
                            nc.tensor.matmul(
                                pst[:, :nhs], xt[:, c, mc * P:(mc + 1) * P],
                                tt[:, c, :nhs],
                                start=(c == 0), stop=(c == nch_d - 1))
                        nc.scalar.copy(sraw[:, mc, :nhs], pst[:, :nhs])
                        if mc == 0:
                            nc.vector.tensor_copy(runmax[:, :nhs], sraw[:, 0, :nhs])
                        else:
                            nc.vector.tensor_max(runmax[:, :nhs], runmax[:, :nhs],
                                                 sraw[:, mc, :nhs])
                        if mc == 3:
                            # flush the previous half's deferred AV/outproj
                            # here: its softmax chain has had a full TT slab
                            # plus half this scores sweep to complete
                            for pre, fn in pending:
                                if pre is not None:
                                    pre()
                                fn()
                            pending.clear()

                    # softmax over m (partition axis x chunk axis)
                    maxb = p_misc1.tile([P, half], F32, tag="maxb")
                    nc.gpsimd.partition_all_reduce(maxb[:, :nhs], runmax[:, :nhs], P,
                                                   bass_isa.ReduceOp.max)
                    a_t = p_a.tile([P, nch_n, half], BF, tag="a")
                    s1 = p_misc1.tile([P, half], F32, tag="s1")
                    for mc in range(nch_n):
                        nc.vector.tensor_sub(sraw[:, mc, :nhs], sraw[:, mc, :nhs],
                                             maxb[:, :nhs])
                        nc.scalar.activation(a_t[:, mc, :nhs], sraw[:, mc, :nhs],
                                             mybir.ActivationFunctionType.Exp)
                        if mc == 0:
                            nc.vector.tensor_copy(s1[:, :nhs], a_t[:, 0, :nhs])
                        else:
                            nc.vector.tensor_add(s1[:, :nhs], s1[:, :nhs],
                                                 a_t[:, mc, :nhs])
                    denb = p_misc1.tile([P, half], F32, tag="maxb", name="denb")
                    nc.gpsimd.partition_all_reduce(denb[:, :nhs], s1[:, :nhs], P,
                                                   bass_isa.ReduceOp.add)

                    # HhT[v, n-half] = sum_m V[m, v] A[m, n] -- deferred (bf16)
                    # recip prologue fires mid-TT-slab (after the 2nd cast) so
                    # it neither blocks the tt casts nor delays the cat mult
                    rbox = {}

                    def emit_recip(denb=denb, nhs=nhs, rbox=rbox):
                        recip = p_misc1.tile([P, half], F32, tag="s1",
                                             name="recip")
                        nc.vector.reciprocal(recip[:dv, :nhs], denb[:dv, :nhs])
                        rbox["recip"] = recip

                    def emit_av(v_both=v_both, a_t=a_t, cat=cat, hl=hl,
                                nho=nho, nhs=nhs, rbox=rbox):
                        psav = ps_v.tile([P, FREE], F32, tag="psv", name="psav")
                        for mc in range(nch_n):
                            nc.tensor.matmul(
                                psav[:dv, :nhs],
                                v_both[:, mc, hl * dv:(hl + 1) * dv],
                                a_t[:, mc, :nhs],
                                start=(mc == 0), stop=(mc == nch_n - 1))
                        recip = rbox["recip"]
                        nc.vector.tensor_mul(cat[hl * dv:(hl + 1) * dv, nho:nho + nhs],
                                             psav[:dv, :nhs], recip[:dv, :nhs])
                    pending.append((emit_recip, emit_av))

                    # ---- output projection rows for this half once all
                    # heads are done (pipelines under the next half's TT)
                    if hl == hloc - 1:
                        def emit_outproj(cat=cat, b=b, nho=nho, nhs=nhs):
                            for ncc in range(nho // P, (nho + nhs) // P):
                                ct = cat[:, ncc * P:(ncc + 1) * P]
                                for (dho, dhs) in _fsplits(d_sz, FREE):
                                    if dho >= ncc * P + P:
                                        # fully masked block: constant fill
                                        for fo in range(0, dhs, FREE // 2):
                                            fs = min(FREE // 2, dhs - fo)
                                            nc.sync.dma_start(
                                                y_ap[b, ncc * P:(ncc + 1) * P,
                                                     dho + fo:dho + fo + fs],
                                                fill_tile[:, :fs])
                                        continue
                                    pst = ps_o.tile([P, FREE], F32, tag="pstr",
                                                    name="psy")
                                    nc.tensor.matmul(pst[:, :dhs], ct,
                                                     wo[:, dho:dho + dhs],
                                                     start=True, stop=True)
                                    # drain in 256-wide pieces alternating two
                                    # small yt tiles and two copy engines so
                                    # nothing idles on the copy->select->DMA
                                    # WAR chain; fully-masked pieces skip the
                                    # copy entirely (constant fill DMA)
                                    for fo in range(0, dhs, FREE // 2):
                                        fs = min(FREE // 2, dhs - fo)
                                        if dho + fo > ncc * P + P - 1:
                                            nc.sync.dma_start(
                                                y_ap[b, ncc * P:(ncc + 1) * P,
                                                     dho + fo:dho + fo + fs],
                                                fill_tile[:, :fs])
                                            continue
                                        pidx = (dho + fo) // (FREE // 2)
                                        yt = p_y.tile([P, FREE // 2], F32,
                                                      tag=f"yt{pidx % 2}",
                                                      name=f"yt{pidx % 2}")
                                        if pidx % 2 == 0:
                                            nc.scalar.copy(yt[:, :fs],
                                                           pst[:, fo:fo + fs])
                                        else:
                                            nc.vector.tensor_copy(
                                                yt[:, :fs], pst[:, fo:fo + fs])
                                        if dho + fo + fs > ncc * P + 1:
                                            # diagonal piece: keep where
                                            # row - col >= 0, else fill
                                            nc.gpsimd.affine_select(
                                                out=yt[:, :fs], in_=yt[:, :fs],
                                                compare_op=mybir.AluOpType.is_ge,
                                                fill=fill,
                                                base=ncc * P - dho - fo,
                                                pattern=[[-1, fs]],
                                                channel_multiplier=1)
                                        nc.sync.dma_start(
                                            y_ap[b, ncc * P:(ncc + 1) * P,
                                                 dho + fo:dho + fo + fs],
                                            yt[:, :fs])
                        pending.append((None, emit_outproj))

        for pre, fn in pending:
            if pre is not None:
                pre()
            fn()
        pending.clear()


def build_program(b_sz=B, n_sz=N, d_sz=D, dv=DV, hloc=HLOC, fill_div=NCORES,
                  num_devices=NCORES):
    nc = bacc.Bacc("TRN2", target_bir_lowering=False, debug=False,
                   num_devices=num_devices)
    hv = hloc * dv
    specs = {
        "xt": ([b_sz, d_sz, n_sz], F32R),
        "wqt": ([hloc, d_sz, d_sz], F32R),
        "wkt": ([hloc, d_sz, d_sz], F32R),
        "wv2": ([d_sz, hv], F32R),
        "wo": ([hv, d_sz], BF),
    }
    ins = {k: nc.dram_tensor(k, shp, dt, kind="ExternalInput").ap()
           for k, (shp, dt) in specs.items()}
    y = nc.dram_tensor("y", [b_sz, n_sz, d_sz], F32, kind="ExternalOutput").ap()
    with tile.TileContext(nc) as tc:
        build_mha_body(tc, ins, y, b_sz=b_sz, n_sz=n_sz, d_sz=d_sz, dv=dv,
                       hloc=hloc, fill_div=fill_div)
    nc.compile()
    return nc


def make_in_maps(X, W_q, W_k, W_v, W_o, ncores=NCORES, hloc=HLOC):
    scale = np.float32(1.0 / np.sqrt(X.shape[2]))
    xt = np.ascontiguousarray(X.transpose(0, 2, 1))
    dvv = W_v.shape[2]
    in_maps = []
    for c in range(ncores):
        hs = slice(c * hloc, (c + 1) * hloc)
        wqt = np.ascontiguousarray((W_q[hs] * scale).transpose(0, 2, 1))
        wkt = np.ascontiguousarray(W_k[hs].transpose(0, 2, 1))
        wv2 = np.ascontiguousarray(
            np.concatenate([W_v[c * hloc + i] for i in range(hloc)], axis=1))
        wo = np.ascontiguousarray(
            W_o[c * hloc * dvv:(c + 1) * hloc * dvv]).astype(bf16)
        in_maps.append({
            "xt": xt, "wqt": wqt, "wkt": wkt, "wv2": wv2, "wo": wo,
        })
    return in_maps


_CACHE = {}


def kernel(X, W_q, W_k, W_v, W_o, _trace=False):
    from concourse.bass_utils import run_bass_kernel_spmd
    X = np.asarray(X, dtype=np.float32)
    W_q = np.asarray(W_q, dtype=np.float32)
    W_k = np.asarray(W_k, dtype=np.float32)
    W_v = np.asarray(W_v, dtype=np.float32)
    W_o = np.asarray(W_o, dtype=np.float32)

    if "nc" not in _CACHE:
        _CACHE["nc"] = build_program()
    nc = _CACHE["nc"]

    in_maps = make_in_maps(X, W_q, W_k, W_v, W_o)
    res = run_bass_kernel_spmd(nc, in_maps, list(range(NCORES)), trace=_trace)
    parts = [r["y"].astype(np.float32) for r in res.results]
    out = parts[0]
    for p in parts[1:]:
        out = out + p
    if _trace:
        _CACHE["last_result"] = res
    return out


# revision 26
# speedup vs baseline: 1.1134x; 1.0135x over previous
"""Trainium2 Bass kernel for the MultiHeadAttention problem.

Math (per head h):
  scores = (X Wq_h) (X Wk_h)^T * scale = X (scale * Wq_h Wk_h^T) X^T
so we precompute M_h = (scale*Wq_h) Wk_h^T once per head (batch independent),
then per batch compute scores directly in the transposed [m, n] orientation so
softmax's reduction axis (m) lands on partitions and A feeds the A@V matmul
without transposes:
  TT[d', n] = sum_d M[d, d'] X^T[d, n]
  scoresT[m, n] = sum_d' X^T[d', m] TT[d', n]
  A = exp(scoresT - colmax) -> HhT[v, n] = sum_m V[m, v] A[m, n] / colsum
Output projection: Y_partial = concatT^T @ Wo_local, with the post-hoc triu
mask filled with LARGE_NEG/8 on every core so the host-side shard-sum yields
exactly LARGE_NEG at masked positions.

Sharding: 16 heads / 8 cores = 2 heads per core, every core handles all 4
batches; host sums the 8 partial outputs (the only cross-core reduction).

Precision: the score path (M, TT, scoresT) and V use single-pass fp32r
matmuls — TRN2's fp32r mode runs at bf16 rate (1 cycle/row for moving dim
>= 256) with ~12-bit mantissa operand precision, giving score errors ~0.3
absolute against softmax top-2 gaps of ~400, so argmax fidelity is preserved
without the old 3-pass bf16 hi/lo splits.  V is computed transposed
(VT = Wv2^T X^T, both heads stacked, moving dim 512) then PE-transposed into
key-major layout to avoid overhead-dominated free=64 matmuls.  The A/V/AV and
output-projection paths run in 1-pass bf16 (~0.5% relative, far inside
tolerance).
"""

import os
import sys

import numpy as np
import ml_dtypes

for _p in ("/opt/trn_rl_repo",):
    if os.path.isdir(_p) and _p not in sys.path:
        sys.path.insert(0, _p)

import concourse.bass as bass
import concourse.tile as tile
from concourse import bacc, bass_isa, mybir

BF = mybir.dt.bfloat16
F32 = mybir.dt.float32
F32R = mybir.dt.float32r
bf16 = ml_dtypes.bfloat16

# Problem constants (hardcoded per contract)
B, N, D, DV, H = 4, 1024, 1024, 64, 16
NCORES = 8
HLOC = H // NCORES  # heads per core
P = 128
FREE = 512  # PSUM free-dim limit for fp32 outputs
LARGE_NEG = -1.0e9


def _fsplits(total, step):
    return [(o, min(step, total - o)) for o in range(0, total, step)]


def build_mha_body(tc, ins, y_ap, b_sz=B, n_sz=N, d_sz=D, dv=DV, hloc=HLOC,
                   fill_div=NCORES):
    """Emit the per-core MHA program into TileContext tc.

    ins: dict of dram APs (f32r unless noted): xt [b, d, n], wqt/wkt
    [hloc, d, d] (wqt pre-scaled), wv2 [d, hloc*dv], wo [hloc*dv, d] (bf16).
    y_ap: [b, n, d] f32 output.
    """
    nc = tc.nc
    nch_d = d_sz // P
    nch_n = n_sz // P
    half = min(FREE, n_sz)
    hv = hloc * dv
    assert hv <= P
    fill = float(LARGE_NEG / fill_div)

    import contextlib
    ctx = contextlib.ExitStack()
    with ctx:
        p_m = ctx.enter_context(tc.tile_pool(name="mpool", bufs=1))
        p_xt = ctx.enter_context(tc.tile_pool(name="xt", bufs=1))
        p_wk = ctx.enter_context(tc.tile_pool(name="wk", bufs=1))
        p_wq = ctx.enter_context(tc.tile_pool(name="wq", bufs=2))
        p_tt = ctx.enter_context(tc.tile_pool(name="tt", bufs=1))
        p_sraw = ctx.enter_context(tc.tile_pool(name="sraw", bufs=1))
        p_a = ctx.enter_context(tc.tile_pool(name="apool", bufs=1))
        p_v = ctx.enter_context(tc.tile_pool(name="vpool", bufs=1))
        p_wv = ctx.enter_context(tc.tile_pool(name="wv", bufs=1))
        p_wo = ctx.enter_context(tc.tile_pool(name="wo", bufs=1))
        p_cat = ctx.enter_context(tc.tile_pool(name="cat", bufs=1))
        p_misc1 = ctx.enter_context(tc.tile_pool(name="misc1", bufs=1))
        p_y = ctx.enter_context(tc.tile_pool(name="yout", bufs=1))
        ps = ctx.enter_context(tc.tile_pool(name="ps", bufs=4, space="PSUM"))
        ps_v = ctx.enter_context(tc.tile_pool(name="psv", bufs=1, space="PSUM"))
        ps_o = ctx.enter_context(tc.tile_pool(name="pso", bufs=3, space="PSUM"))

        # Wo resident (local head rows), natural layout [hv, d], bf16
        wo = p_wo.tile([hv, d_sz], BF, tag="wo")
        nc.sync.dma_start(wo[:], ins["wo"][:])
        # Wv for both local heads stacked column-wise: [P, nch_d, hv] f32r
        wv2 = p_wv.tile([P, nch_d, hv], F32R, tag="wv2")
        nc.sync.dma_start(wv2[:], ins["wv2"].rearrange("(c p) v -> p c v", p=P))
        fill_tile = p_wo.tile([P, FREE // 2], F32, tag="fill_tile")
        nc.gpsimd.memset(fill_tile[:], fill)
        # identity for PE transposes: start from ones, keep only the diagonal
        ident = p_wo.tile([P, P], F32, tag="ident")
        nc.gpsimd.memset(ident[:], 1.0)
        nc.gpsimd.affine_select(
            out=ident[:], in_=ident[:], compare_op=mybir.AluOpType.is_equal,
            fill=0.0, base=0, pattern=[[-1, P]], channel_multiplier=1)

        concat_tiles = {}
        vb_tiles = {}
        pending = []  # deferred AV/out-proj emitters, flushed after the next
        # slab's TT matmuls so the PE never waits on a softmax chain

        # visit order over (hl, b): snake so head boundaries reuse X^T
        visits = []
        for hl in range(hloc):
            border = range(b_sz) if hl % 2 == 0 else range(b_sz - 1, -1, -1)
            visits.extend((hl, b) for b in border)
        xt_seq = []  # deduped consecutive batch sequence (positions)
        vis_pos = []
        for _, b in visits:
            if not xt_seq or xt_seq[-1] != b:
                xt_seq.append(b)
            vis_pos.append(len(xt_seq) - 1)
        # explicit double-buffering: position i lives in buffer i%2, which
        # under the snake order exactly matches buffer liveness
        xt_state = {}  # buf -> owning batch
        xt_live = {}   # batch -> tile

        def xt_load(pos):
            b = xt_seq[pos]
            buf = pos % 2
            if xt_state.get(buf) == b:
                return xt_live[b]
            t = p_xt.tile([P, nch_d, n_sz], F32R, tag=f"xt{buf}",
                          name=f"xt{b}p{pos}")
            for c in range(nch_d):
                nc.sync.dma_start(t[:, c, :],
                                  ins["xt"][b][c * P:(c + 1) * P, :])
            old = xt_state.get(buf)
            if old is not None:
                xt_live.pop(old, None)
            xt_state[buf] = b
            xt_live[b] = t
            return t

        wkf_tiles = {}

        def ensure_wkf(h):
            if h in wkf_tiles:
                return wkf_tiles[h]
            t = p_wk.tile([P, nch_d, d_sz], F32R, tag="wkf", name=f"wkf{h}")
            for c in range(nch_d):
                nc.sync.dma_start(t[:, c, :],
                                  ins["wkt"][h][c * P:(c + 1) * P, :])
            wkf_tiles[h] = t
            return t

        vi = 0
        for hl in range(hloc):
            # ---- M phase: M[d, d'] = sum_e WqT[e, d] WkT[e, d'] (f32r)
            # Wk fully resident per head; each weight byte is DMA'd exactly once.
            m_t = p_m.tile([P, nch_d, d_sz], F32R, tag="m")
            wkf = ensure_wkf(hl)
            for dc in range(nch_d):
                wq = p_wq.tile([P, nch_d, P], F32R, tag="wq")
                nc.sync.dma_start(
                    wq[:],
                    ins["wqt"][hl][:, dc * P:(dc + 1) * P].rearrange(
                        "(c p) f -> p c f", p=P))
                for (dpo, dps) in _fsplits(d_sz, FREE):
                    pst = ps.tile([P, FREE], F32, tag="ps")
                    for e in range(nch_d):
                        nc.tensor.matmul(pst[:, :dps], wq[:, e, :],
                                         wkf[:, e, dpo:dpo + dps],
                                         start=(e == 0), stop=(e == nch_d - 1))
                    nc.scalar.copy(m_t[:, dc, dpo:dpo + dps], pst[:, :dps])

            # ---- attention phase (snake order so the head boundary reuses
            # the resident X^T tile of the last batch)
            border = range(b_sz) if hl % 2 == 0 else range(b_sz - 1, -1, -1)
            for b in border:
                pos = vis_pos[vi]
                vi += 1
                xt = xt_load(pos)
                if hl == hloc - 1:
                    # the fully-masked y regions are constants: write them now
                    # so their DMAs spread over this batch's compute instead
                    # of trailing the final flush
                    for ncc in range(nch_n):
                        for fo in range(0, d_sz, FREE // 2):
                            fs = min(FREE // 2, d_sz - fo)
                            if fo > ncc * P + P - 1:
                                nc.sync.dma_start(
                                    y_ap[b, ncc * P:(ncc + 1) * P, fo:fo + fs],
                                    fill_tile[:, :fs])
                # prefetch the next position's X^T so its DMA dispatches
                # before this batch's long compute phase
                if pos + 1 < len(xt_seq):
                    xt_load(pos + 1)
                if b == (border[-1] if hl % 2 == 0 else 0) and hl + 1 < hloc:
                    ensure_wkf(hl + 1)

                if b not in concat_tiles:
                    concat_tiles[b] = p_cat.tile([hv, n_sz], BF, tag=f"cat{b}",
                                                 name=f"cat{b}")
                cat = concat_tiles[b]

                # ---- V for both heads, once per batch (at first head):
                # VT[v2, n] = sum_d Wv2[d, v2] X^T[d, n]  (f32r, moving dim 512)
                # then PE-transpose 128x128 blocks into key-major v_both (bf16).
                # Emission is deferred until after the first TT slab so its PE
                # work covers the previous half's softmax chain and its DVE
                # copies queue behind the tt casts.
                emit_v = None
                if b not in vb_tiles:
                    vb_tiles[b] = p_v.tile([P, nch_n, hv], BF, tag=f"vb{b}",
                                           name=f"vb{b}")

                    def emit_v(vb=vb_tiles[b], xt=xt):
                        for (vnho, vnhs) in _fsplits(n_sz, half):
                            pvt = ps_v.tile([P, FREE], F32, tag="psv")
                            for c in range(nch_d):
                                nc.tensor.matmul(pvt[:hv, :vnhs], wv2[:, c, :],
                                                 xt[:, c, vnho:vnho + vnhs],
                                                 start=(c == 0),
                                                 stop=(c == nch_d - 1))
                            vt_sb = p_misc1.tile([P, FREE], F32, tag="s1",
                                                 name="vt_sb")
                            nc.vector.tensor_copy(vt_sb[:hv, :vnhs],
                                                  pvt[:hv, :vnhs])
                            for j in range(vnhs // P):
                                ptr = ps_o.tile([P, FREE], F32, tag="pstr")
                                nc.tensor.transpose(
                                    ptr[:, :hv],
                                    vt_sb[:hv, j * P:(j + 1) * P],
                                    ident[:hv, :hv])
                                nc.vector.tensor_copy(vb[:, vnho // P + j, :],
                                                      ptr[:, :hv])
                v_both = vb_tiles[b]

                # the very last visit runs its halves high-to-low so the
                # final (uncovered) outproj flush is the cheap ncc 0-3 one
                halves = _fsplits(n_sz, half)
                if vi == len(visits):
                    halves = halves[::-1]
                for (nho, nhs) in halves:
                    # TT[d', n-half] = sum_d M[d, d'] XT[d, n]  (f32r)
                    tt = p_tt.tile([P, nch_d, half], F32R, tag="tt")
                    for dp in range(nch_d):
                        pst = ps.tile([P, FREE], F32, tag="ps")
                        for dc in range(nch_d):
                            nc.tensor.matmul(
                                pst[:, :nhs], m_t[:, dc, dp * P:(dp + 1) * P],
                                xt[:, dc, nho:nho + nhs],
                                start=(dc == 0), stop=(dc == nch_d - 1))
                        if dp % 2 == 0:
                            nc.vector.tensor_copy(tt[:, dp, :nhs], pst[:, :nhs])
                        else:
                            nc.scalar.copy(tt[:, dp, :nhs], pst[:, :nhs])

                    if emit_v is not None:
                        emit_v()
                        emit_v = None

                    # scoresT[m, n-half]  (f32r)
                    sraw = p_sraw.tile([P, nch_n, half], F32, tag="sraw")
                    runmax = p_misc1.tile([P, half], F32, tag="runmax")
                    for mc in range(nch_n):
                        pst = ps.tile([P, FREE], F32, tag="ps")
                        for c in range(nch_d):
                            nc.tensor.matmul(
                                pst[:, :nhs], xt[:, c, mc * P:(mc + 1) * P],
                                tt[:, c, :nhs],
                                start=(c == 0), stop=(c == nch_d - 1))
                        nc.scalar.copy(sraw[:, mc, :nhs], pst[:, :nhs])
                        if mc == 0:
                            nc.vector.tensor_copy(runmax[:, :nhs], sraw[:, 0, :nhs])
                        else:
                            nc.vector.tensor_max(runmax[:, :nhs], runmax[:, :nhs],
                                                 sraw[:, mc, :nhs])
                        if mc == 3:
                            # flush the previous half's deferred AV/outproj
                            # here: its softmax chain has had a full TT slab
                            # plus half this scores sweep to complete
                            for pre, fn in pending:
                                if pre is not None:
                                    pre()
                                fn()
                            pending.clear()

                    # softmax over m (partition axis x chunk axis)
                    maxb = p_misc1.tile([P, half], F32, tag="maxb")
                    nc.gpsimd.partition_all_reduce(maxb[:, :nhs], runmax[:, :nhs], P,
                                                   bass_isa.ReduceOp.max)
                    a_t = p_a.tile([P, nch_n, half], BF, tag="a")
                    s1 = p_misc1.tile([P, half], F32, tag="s1")
                    for mc in range(nch_n):
                        nc.vector.tensor_sub(sraw[:, mc, :nhs], sraw[:, mc, :nhs],
                                             maxb[:, :nhs])
                        nc.scalar.activation(a_t[:, mc, :nhs], sraw[:, mc, :nhs],
                                             mybir.ActivationFunctionType.Exp)
                        if mc == 0:
                            nc.vector.tensor_copy(s1[:, :nhs], a_t[:, 0, :nhs])
                        else:
                            nc.vector.tensor_add(s1[:, :nhs], s1[:, :nhs],
                                                 a_t[:, mc, :nhs])
                    denb = p_misc1.tile([P, half], F32, tag="maxb", name="denb")
                    nc.gpsimd.partition_all_reduce(denb[:, :nhs], s1[:, :nhs], P,
                                                   bass_isa.ReduceOp.add)

                    # HhT[v, n-half] = sum_m V[m, v] A[m, n] -- deferred (bf16)
                    # recip prologue fires mid-TT-slab (after the 2nd cast) so
                    # it neither blocks the tt casts nor delays the cat mult
                    rbox = {}

                    def emit_recip(denb=denb, nhs=nhs, rbox=rbox):
                        recip = p_misc1.tile([P, half], F32, tag="s1",
                                             name="recip")
                        nc.vector.reciprocal(recip[:dv, :nhs], denb[:dv, :nhs])
                        rbox["recip"] = recip

                    def emit_av(v_both=v_both, a_t=a_t, cat=cat, hl=hl,
                                nho=nho, nhs=nhs, rbox=rbox):
                        psav = ps_v.tile([P, FREE], F32, tag="psv", name="psav")
                        for mc in range(nch_n):
                            nc.tensor.matmul(
                                psav[:dv, :nhs],
                                v_both[:, mc, hl * dv:(hl + 1) * dv],
                                a_t[:, mc, :nhs],
                                start=(mc == 0), stop=(mc == nch_n - 1))
                        recip = rbox["recip"]
                        nc.vector.tensor_mul(cat[hl * dv:(hl + 1) * dv, nho:nho + nhs],
                                             psav[:dv, :nhs], recip[:dv, :nhs])
                    pending.append((emit_recip, emit_av))

                    # ---- output projection rows for this half once all
                    # heads are done (pipelines under the next half's TT)
                    if hl == hloc - 1:
                        def emit_outproj(cat=cat, b=b, nho=nho, nhs=nhs):
                            for ncc in range(nho // P, (nho + nhs) // P):
                                ct = cat[:, ncc * P:(ncc + 1) * P]
                                for (dho, dhs) in _fsplits(d_sz, FREE):
                                    if dho >= ncc * P + P:
                                        continue  # fully masked: filled early
                                    pst = ps_o.tile([P, FREE], F32, tag="pstr",
                                                    name="psy")
                                    nc.tensor.matmul(pst[:, :dhs], ct,
                                                     wo[:, dho:dho + dhs],
                                                     start=True, stop=True)
                                    # drain in 256-wide pieces alternating two
                                    # small yt tiles and two copy engines so
                                    # nothing idles on the copy->select->DMA
                                    # WAR chain; fully-masked pieces skip the
                                    # copy entirely (constant fill DMA)
                                    for fo in range(0, dhs, FREE // 2):
                                        fs = min(FREE // 2, dhs - fo)
                                        if dho + fo > ncc * P + P - 1:
                                            continue  # filled early
                                        pidx = (dho + fo) // (FREE // 2)
                                        yt = p_y.tile([P, FREE // 2], F32,
                                                      tag=f"yt{pidx % 2}",
                                                      name=f"yt{pidx % 2}")
                                        if pidx % 2 == 0:
                                            nc.scalar.copy(yt[:, :fs],
                                                           pst[:, fo:fo + fs])
                                        else:
                                            nc.vector.tensor_copy(
                                                yt[:, :fs], pst[:, fo:fo + fs])
                                        if dho + fo + fs > ncc * P + 1:
                                            # diagonal piece: keep where
                                            # row - col >= 0, else fill
                                            nc.gpsimd.affine_select(
                                                out=yt[:, :fs], in_=yt[:, :fs],
                                                compare_op=mybir.AluOpType.is_ge,
                                                fill=fill,
                                                base=ncc * P - dho - fo,
                                                pattern=[[-1, fs]],
                                                channel_multiplier=1)
                                        nc.sync.dma_start(
                                            y_ap[b, ncc * P:(ncc + 1) * P,
                                                 dho + fo:dho + fo + fs],
                                            yt[:, :fs])
                        pending.append((None, emit_outproj))

        for pre, fn in pending:
            if pre is not None:
                pre()
            fn()
        pending.clear()


def build_program(b_sz=B, n_sz=N, d_sz=D, dv=DV, hloc=HLOC, fill_div=NCORES,
                  num_devices=NCORES):
    nc = bacc.Bacc("TRN2", target_bir_lowering=False, debug=False,
                   num_devices=num_devices)
    hv = hloc * dv
    specs = {
        "xt": ([b_sz, d_sz, n_sz], F32R),
        "wqt": ([hloc, d_sz, d_sz], F32R),
        "wkt": ([hloc, d_sz, d_sz], F32R),
        "wv2": ([d_sz, hv], F32R),
        "wo": ([hv, d_sz], BF),
    }
    ins = {k: nc.dram_tensor(k, shp, dt, kind="ExternalInput").ap()
           for k, (shp, dt) in specs.items()}
    y = nc.dram_tensor("y", [b_sz, n_sz, d_sz], F32, kind="ExternalOutput").ap()
    with tile.TileContext(nc) as tc:
        build_mha_body(tc, ins, y, b_sz=b_sz, n_sz=n_sz, d_sz=d_sz, dv=dv,
                       hloc=hloc, fill_div=fill_div)
    nc.compile()
    return nc


def make_in_maps(X, W_q, W_k, W_v, W_o, ncores=NCORES, hloc=HLOC):
    scale = np.float32(1.0 / np.sqrt(X.shape[2]))
    xt = np.ascontiguousarray(X.transpose(0, 2, 1))
    dvv = W_v.shape[2]
    in_maps = []
    for c in range(ncores):
        hs = slice(c * hloc, (c + 1) * hloc)
        wqt = np.ascontiguousarray((W_q[hs] * scale).transpose(0, 2, 1))
        wkt = np.ascontiguousarray(W_k[hs].transpose(0, 2, 1))
        wv2 = np.ascontiguousarray(
            np.concatenate([W_v[c * hloc + i] for i in range(hloc)], axis=1))
        wo = np.ascontiguousarray(
            W_o[c * hloc * dvv:(c + 1) * hloc * dvv]).astype(bf16)
        in_maps.append({
            "xt": xt, "wqt": wqt, "wkt": wkt, "wv2": wv2, "wo": wo,
        })
    return in_maps


_CACHE = {}


def kernel(X, W_q, W_k, W_v, W_o, _trace=False):
    from concourse.bass_utils import run_bass_kernel_spmd
    X = np.asarray(X, dtype=np.float32)
    W_q = np.asarray(W_q, dtype=np.float32)
    W_k = np.asarray(W_k, dtype=np.float32)
    W_v = np.asarray(W_v, dtype=np.float32)
    W_o = np.asarray(W_o, dtype=np.float32)

    if "nc" not in _CACHE:
        _CACHE["nc"] = build_program()
    nc = _CACHE["nc"]

    in_maps = make_in_maps(X, W_q, W_k, W_v, W_o)
    res = run_bass_kernel_spmd(nc, in_maps, list(range(NCORES)), trace=_trace)
    parts = [r["y"].astype(np.float32) for r in res.results]
    out = parts[0]
    for p in parts[1:]:
        out = out + p
    if _trace:
        _CACHE["last_result"] = res
    return out
